# revision 26
# baseline (speedup 1.0000x reference)
"""Trainium2 Bass kernel for nn_CPDP_AM_net_SGBS (3-layer MHA decoder step), v4.

Contract: kernel(**inputs) takes FULL inputs (B=256) and returns the FULL
output (256, 512).  Internally shards the batch dim across 8 NeuronCores
(32 batches/core), data-parallel, no cross-core communication.

v4 strategy (memory-regime), building on v2 (fp8 host quantization,
host-pre-transposed K, block-diag query scores, y-form AV) and v3
(mask-based row compaction):

  - The ~30% of K/V rows the mask forbids are dropped host-side: each
    batch's legal rows are gathered front-first.  A global batch
    permutation (undone in the host-side scatter) deals the largest-count
    batches round-robin across cores and into group 0 of each core, so
    GROUP 0 runs with capacity NPAD0=416 (3 full 128-row n-chunks + one
    32-row V-tail matmul) while GROUPS 1..7 run with capacity NPADR=384
    (3 full chunks, no tail).  Seed-0 inputs: max legal rows 388, five
    batches exceed 384 -> easily placed.  Any input that does not fit
    falls back to uniform npad 512 (same code path, always correct).
  - Pad slots carry K=V=0, so their scores are exactly 0 and their
    exp contributes exactly 1.0 (layers 0/1) / exp(-CLIP) (layer 2) to
    the softmax denominator; the exact per-batch count is subtracted
    on-device (nzc/nzc2 via DVE) — no -1e9 bias matmuls at all.
  - softmax without max-subtraction (logits are tiny): ACT exp(scale=cs)
    with fused row-sum, DVE reciprocal, 1/Z deferred into the head-diag
    extraction (scalar_tensor_tensor with selY).
  - w'^T for AV via PE transposes + ACT fp8 cast (default) or DVE
    square-block transposes in bf16 (dve_t=True).
  - head-diag slot-row sums as 4 tiny PE matmuls vs a 0/1 selector.
  - projections (W0, Wq) as accumulated matmuls with host-scaled
    transposed bf16 weights, emitted per half (halves=2) so they overlap
    the second half of each layer's groups.

HBM traffic per core: ~31.9 MB (vs 42 MB v2, 168 MB plain fp32).

Scale ledger (host <-> device):
  K8 = e4(16*K), V8 = e4(16*V), q0_8 = e4(16*q0), w8 = e4(exp(logit)) ~ 1
  (unnormalized; 1/Z folded into the extraction via scalar_tensor_tensor)
  attn_dev = 16*attn ; W0h = (100/16)*W0^T -> q1_dev = 100*q1 (same for
  q2); Wqh = Wq^T -> qf_dev = 100*qf.
  exp scales: cs0 = 1/(16*16*sqrt(32)), cs1 = 1/(100*16*sqrt(32));
  tanh scale: ct = 1/(100*16*sqrt(512)).
"""

import sys

if "/opt/trn_rl_repo" not in sys.path:
    sys.path.insert(0, "/opt/trn_rl_repo")

import numpy as np

import concourse.bass as bass
import concourse.tile as tile
import concourse.mybir as mybir

F32 = mybir.dt.float32
BF16 = mybir.dt.bfloat16
FP8 = mybir.dt.float8e4

N_CORES = 8
B = 256
N = 512
D = 512
H = 16
DH = 32
DC = 4                # d chunks of 128
NPAD0 = 416           # group-0 capacity (13*32)
NPADR = 384           # groups 1+ capacity (3*128)
CLIP = 10.0

SK = 16.0             # fp8 scale for K, V, q0
SQ = 100.0            # device scale of q1/q2/qf
CS0 = 1.0 / (SK * SK * np.sqrt(DH))
CS1 = 1.0 / (SQ * SK * np.sqrt(DH))
CT = 1.0 / (SQ * SK * np.sqrt(D))


def _hoist_excess_matmul_waits(nc, keep=1):
    """walrus limits self-loading 4-byte matmuls (fp32/fp32r/transpose) to a
    single sync wait on the S3_LW struct.  Hoist excess waits onto a
    standalone PE EventSemaphore inserted right before the matmul — same
    engine, so per-engine program order makes it equivalent."""
    for fn in nc.m.functions:
        for blk in fn.blocks:
            il = blk.instructions
            i = 0
            while i < len(il):
                inst = il[i]
                si = inst.sync_info
                if (type(inst).__name__ != "InstEventSemaphore"
                        and si is not None
                        and si.on_wait and len(si.on_wait) > keep):
                    moved = list(si.on_wait[:-keep]) if keep else list(si.on_wait)
                    kept = list(si.on_wait[-keep:]) if keep else []
                    for j, w in enumerate(moved):
                        wi = mybir.InstEventSemaphore(
                            name=f"{inst.name}-hw{j}",
                            ins=[], outs=[],
                            sync_info=mybir.SyncInfo(on_wait=[w], on_update=[]),
                        )
                        wi.engine = inst.engine
                        nc.register_instruction(wi)
                        il.insert(i, wi)
                        i += 1
                    inst.sync_info = mybir.SyncInfo(
                        on_wait=kept, on_update=list(si.on_update)
                    )
                i += 1


def build_nc(b_core=32, npad0=NPAD0, npadr=NPADR, reps=1, dma_only=False,
             kv_bufs=5, k2_bufs=3, compute_only=False, staggered=False,
             unroll=False, dve_t=False, halves=2):
    """Build the single-core Bass program for a [b_core]-batch shard with
    group-0 n-capacity npad0 (tail = npad0 % 128 via a resident V-tail
    tile) and npadr for the remaining groups (npadr % 128 must be 0,
    or npadr == npad0)."""
    groups = b_core // 4
    nfull0 = npad0 // 128
    ntail0 = npad0 - 128 * nfull0     # 0 or a multiple of 32
    nfullr = npadr // 128
    assert npadr == npad0 or npadr % 128 == 0
    assert nfull0 == nfullr or ntail0 == 0
    nc = bass.Bass()

    def np_g(g):
        return npad0 if g == 0 else npadr

    # K^T chunks (idx 4k+e: partition p = d within chunk e of batch k,
    # free = compacted n); group 0 vs rest have different n capacity
    kt0 = nc.declare_dram_parameter("kt0", [2, 128, 16, npad0], FP8,
                                    isOutput=False)
    ktr = nc.declare_dram_parameter("ktr", [2 * (groups - 1), 128, 16, npadr],
                                    FP8, isOutput=False)
    # V full chunks (idx k*nfull+c: partition p = n within chunk c of
    # batch k, free = d); same shape for both classes (nfull equal)
    v0 = nc.declare_dram_parameter("v0", [2, 128, 4 * nfull0, D], FP8,
                                   isOutput=False)
    vr = nc.declare_dram_parameter("vr", [2 * (groups - 1), 128, 4 * nfullr, D],
                                   FP8, isOutput=False)
    if ntail0:
        vt = nc.declare_dram_parameter("vt", [ntail0, 2, 4, D], FP8,
                                       isOutput=False)
    k20 = nc.declare_dram_parameter("k20", [128, 16, npad0], FP8,
                                    isOutput=False)
    k2r = nc.declare_dram_parameter("k2r", [groups - 1, 128, 16, npadr], FP8,
                                    isOutput=False)
    qbd0 = nc.declare_dram_parameter("qbd0", [128, DC, b_core * 32], FP8,
                                     isOutput=False)
    w0t = nc.declare_dram_parameter("w0t", [128, DC, D], BF16, isOutput=False)
    wqt = nc.declare_dram_parameter("wqt", [128, DC, D], BF16, isOutput=False)
    b0t = nc.declare_dram_parameter("b0t", [128, DC], F32, isOutput=False)
    bqt = nc.declare_dram_parameter("bqt", [128, DC], F32, isOutput=False)
    nzc = nc.declare_dram_parameter("nzc", [128, groups], F32, isOutput=False)
    nzc2 = nc.declare_dram_parameter("nzc2", [128, groups], F32, isOutput=False)
    slotsel = nc.declare_dram_parameter("slotsel", [128, 4], BF16, isOutput=False)
    sely = nc.declare_dram_parameter("sely", [128, D], F32, isOutput=False)
    ident = nc.declare_dram_parameter("ident", [128, 128], F32, isOutput=False)
    out = nc.declare_dram_parameter("out", [b_core, npad0], F32, isOutput=True)

    with tile.TileContext(nc) as tc:
        with (
            tc.tile_pool(name="singles", bufs=1) as singles,
            tc.tile_pool(name="kvpool", bufs=kv_bufs) as kvpool,
            tc.tile_pool(name="k2pool", bufs=k2_bufs) as k2pool,
            tc.tile_pool(name="work", bufs=3) as work,
            tc.tile_pool(name="small", bufs=8) as small,
            tc.tile_pool(name="p_s", bufs=2, space="PSUM") as p_s,
            tc.tile_pool(name="p_y", bufs=2, space="PSUM") as p_y,
            tc.tile_pool(name="p_wt", bufs=1, space="PSUM") as p_wt,
            tc.tile_pool(name="p_zt", bufs=2, space="PSUM") as p_zt,
            tc.tile_pool(name="p_q", bufs=1, space="PSUM") as p_q,
        ):
            # ---- constants / weights ----
            sb_qbd = singles.tile([128, DC, b_core * 32], FP8)
            nc.sync.dma_start(sb_qbd[:], qbd0[:])
            if ntail0:
                sb_vt = singles.tile([ntail0, 2, 4, D], FP8)
                nc.sync.dma_start(sb_vt[:], vt[:])
            sb_w0t = singles.tile([128, DC, D], BF16)
            nc.sync.dma_start(sb_w0t[:], w0t[:])
            sb_wqt = singles.tile([128, DC, D], BF16)
            nc.sync.dma_start(sb_wqt[:], wqt[:])
            sb_b0t = singles.tile([128, DC], F32)
            nc.sync.dma_start(sb_b0t[:], b0t[:])
            sb_bqt = singles.tile([128, DC], F32)
            nc.sync.dma_start(sb_bqt[:], bqt[:])
            sb_nzc = singles.tile([128, groups], F32)
            nc.sync.dma_start(sb_nzc[:], nzc[:])
            sb_nzc2 = singles.tile([128, groups], F32)
            nc.sync.dma_start(sb_nzc2[:], nzc2[:])
            sb_slotsel = singles.tile([128, 4], BF16)
            nc.sync.dma_start(sb_slotsel[:], slotsel[:])
            sb_selyf = singles.tile([128, D], F32)
            nc.sync.dma_start(sb_selyf[:], sely[:])
            sb_ident = singles.tile([128, 128], F32)
            nc.sync.dma_start(sb_ident[:], ident[:])
            sb_nclip = singles.tile([128, 1], F32)
            nc.vector.memset(sb_nclip[:], -CLIP)
            sb_identb = singles.tile([128, 128], BF16)
            nc.vector.tensor_copy(sb_identb[:], sb_ident[:])

            # qf_pad zeroing is off the critical path: do it first (once —
            # unrolled reps rewrite the same lanes each trip)
            qf_pad = singles.tile([128, DC, b_core * 32], FP8)
            nc.vector.memset(qf_pad[:], 0.0)
            qf_v = qf_pad.rearrange("p e (b j) -> p e b j", j=32)

            def softmax_e(ps_s, cs, g, npg):
                """psum scores [128, npg] -> unnormalized e [128, npg] SBUF
                + rz = 1/sum(e) over legal slots; pad slots contribute
                exp(0)=1 each, subtracted exactly via nzc."""
                zsum = small.tile([128, 1], F32, tag="zsum")
                e_t = work.tile([128, npad0], BF16, tag="e_t")
                nc.scalar.activation(
                    e_t[:, :npg], ps_s[:, :npg],
                    mybir.ActivationFunctionType.Exp,
                    scale=cs, accum_out=zsum[:],
                )
                zc = small.tile([128, 1], F32, tag="zc")
                nc.vector.tensor_scalar_add(zc[:], zsum[:], sb_nzc[:, g:g + 1])
                rz = small.tile([128, 1], F32, tag="rz")
                nc.vector.reciprocal(rz[:], zc[:])
                return e_t, rz

            def projection(attn_h, wt, bt, tag, hb, out_dt=F32):
                """q_nextT [128, DC(j), hb] = W^T @ attn^T + bias, for a
                half-layer batch slice (hb batches)."""
                ps_q = p_q.tile([128, DC, hb], F32, tag="ps_q")
                for jc in range(DC):
                    for ic in range(DC):
                        nc.tensor.matmul(
                            ps_q[:, jc, :],
                            wt[:, ic, 128 * jc:128 * jc + 128],
                            attn_h[:, ic, :],
                            start=(ic == 0), stop=(ic == DC - 1),
                        )
                qt = work.tile([128, DC, hb], out_dt, tag=tag)
                for jc in range(DC):
                    nc.vector.tensor_scalar_add(
                        qt[:, jc, :], ps_q[:, jc, :], bt[:, jc:jc + 1]
                    )
                return qt

            def fill_qbd_half(qt, h, hb):
                """Overwrite the block-diagonal of sb_qbd for batches
                [h*hb, (h+1)*hb) from qt [128, DC, hb] (f32 -> fp8 cast;
                split across DVE and ACT)."""
                qbd_v = sb_qbd.rearrange("p e (b j) -> p e b j", j=32)
                for e in range(DC):
                    for g in range(4):
                        dst = qbd_v[32 * g:32 * g + 32, e,
                                    h * hb:(h + 1) * hb, 4 * e + g]
                        src = qt[32 * g:32 * g + 32, e, :]
                        if (e + g) % 2 == 0:
                            nc.vector.tensor_copy(dst, src)
                        else:
                            nc.scalar.copy(dst, src)

            def _emit_body():
                gph = groups // halves          # groups per half
                hb = b_core // halves           # batches per half

                def boundary(l, attn_h, h):
                    """Projection + query-constant fill for half h of layer l;
                    emitted as soon as that half's groups are done."""
                    if l == 0:
                        qt1 = projection(attn_h, sb_w0t, sb_b0t, "qt1", hb)
                        fill_qbd_half(qt1, h, hb)
                    else:
                        q2t = projection(attn_h, sb_w0t, sb_b0t, "qt2", hb,
                                         out_dt=BF16)
                        qft = projection(q2t, sb_wqt, sb_bqt, "qft", hb)
                        for e in range(DC):
                            nc.vector.tensor_copy(
                                qf_v[:, e, h * hb:(h + 1) * hb, 0],
                                qft[:, e, :],
                            )

                # ================= layers 0, 1 =================
                for l in range(2):
                    cs = CS0 if l == 0 else CS1
                    attn_hs = [work.tile([128, DC, hb], BF16, tag=f"attn{h}",
                                         name=f"attn{h}")
                               for h in range(halves)]
                    psat_hs = [p_zt.tile([128, DC, hb], F32, tag="ps_at",
                                         name=f"psat{h}")
                               for h in range(halves)]
                    for g in range(groups):
                        npg = np_g(g)
                        nfg = nfull0 if g == 0 else nfullr
                        ntg = ntail0 if g == 0 else 0
                        attn_sb = attn_hs[g // gph]
                        gh = g % gph            # group index within the half
                        if compute_only:
                            if g == 0 and l == 0:
                                kt_c = singles.tile([128, 16, npad0], FP8)
                                nc.sync.dma_start(kt_c[:], kt0[0])
                                v_c = singles.tile([128, 4 * nfull0, D], FP8)
                                nc.sync.dma_start(v_c[:], v0[0])
                            kt, v = kt_c, v_c
                        elif g == 0:
                            kt = kvpool.tile([128, 16, npad0], FP8, tag="kt0",
                                             bufs=2)
                            nc.sync.dma_start(kt[:], kt0[l])
                            v = kvpool.tile([128, 4 * nfull0, D], FP8,
                                            tag="v0", bufs=2)
                            nc.sync.dma_start(v[:], v0[l])
                        else:
                            kt = kvpool.tile([128, 16, npadr], FP8, tag="ktr")
                            nc.sync.dma_start(kt[:],
                                              ktr[l * (groups - 1) + g - 1])
                            v = kvpool.tile([128, 4 * nfullr, D], FP8,
                                            tag="vr")
                            nc.sync.dma_start(v[:],
                                              vr[l * (groups - 1) + g - 1])
                        # scores: 4 batches col-tiled, wave-major over e
                        ps_s = p_s.tile([128, npad0], F32, tag="ps_s")
                        for e in range(DC):
                            for k in range(4):
                                b = 4 * g + k
                                nc.tensor.matmul(
                                    ps_s[32 * k:32 * k + 32, :npg],
                                    sb_qbd[:, e, 32 * b:32 * b + 32],
                                    kt[:, 4 * k + e, :],
                                    start=(e == 0),
                                    stop=(e == DC - 1),
                                    tile_position=(0, 32 * k),
                                )
                        e_t, rz = softmax_e(ps_s, cs, g, npg)
                        # w'^T: PE transposes + ACT fp8 cast, or DVE
                        # square-block transposes (bf16)
                        if dve_t:
                            wt8 = work.tile([128, 4, 128], BF16, tag="wt8")
                            for c in range(nfg):
                                nc.vector.transpose(
                                    wt8[:, c, :], e_t[:, 128 * c:128 * c + 128]
                                )
                            for t in range(ntg // 32):
                                for k in range(4):
                                    nc.vector.transpose(
                                        wt8[32 * t:32 * t + 32, nfg,
                                            32 * k:32 * k + 32],
                                        e_t[32 * k:32 * k + 32,
                                            128 * nfg + 32 * t:
                                            128 * nfg + 32 * t + 32],
                                    )
                        else:
                            pwt = p_wt.tile([128, 4, 128], BF16, tag="pwt")
                            for c in range(nfg):
                                nc.tensor.transpose(
                                    pwt[:, c, :], e_t[:, 128 * c:128 * c + 128],
                                    sb_identb[:]
                                )
                            if ntg:
                                nc.tensor.transpose(
                                    pwt[:ntg, nfg, :],
                                    e_t[:, 128 * nfg:npg],
                                    sb_identb[:]
                                )
                            wt8 = work.tile([128, 4, 128], FP8, tag="wt8")
                            nc.scalar.copy(wt8[:, :nfg, :], pwt[:, :nfg, :])
                            if ntg:
                                nc.scalar.copy(wt8[:ntg, nfg, :],
                                               pwt[:ntg, nfg, :])
                        # AV y-form: 4 batches col-tiled, wave-major over c
                        ps_y = p_y.tile([128, D], F32, tag="ps_y")
                        for c in range(nfg):
                            for k in range(4):
                                nc.tensor.matmul(
                                    ps_y[32 * k:32 * k + 32, :],
                                    wt8[:, c, 32 * k:32 * k + 32],
                                    v[:, k * nfg + c, :],
                                    start=(c == 0),
                                    stop=(ntg == 0 and c == nfg - 1),
                                    tile_position=(0, 32 * k),
                                )
                        if ntg:
                            for k in range(4):
                                nc.tensor.matmul(
                                    ps_y[32 * k:32 * k + 32, :],
                                    wt8[:ntg, nfg, 32 * k:32 * k + 32],
                                    sb_vt[:, l, k, :],
                                    start=False, stop=True,
                                    tile_position=(0, 32 * k),
                                )
                        # head-diag extraction with fused 1/Z normalization:
                        # zt = (Y * rz) * selY  -> attn^T[:, :, b]
                        zt = work.tile([128, D], BF16, tag="zt")
                        nc.vector.scalar_tensor_tensor(
                            zt[:], ps_y[:], rz[:], sb_selyf[:],
                            op0=mybir.AluOpType.mult, op1=mybir.AluOpType.mult,
                        )
                        # slot-row sums as 4 tiny PE matmuls: zt chunk as
                        # stationary, 0/1 slot selector streaming ->
                        # attn^T[:, c, 4 batches] straight into PSUM
                        ps_at = psat_hs[g // gph]
                        for c in range(DC):
                            nc.tensor.matmul(
                                ps_at[:, c, 4 * gh:4 * gh + 4],
                                zt[:, 128 * c:128 * c + 128],
                                sb_slotsel[:],
                                start=True, stop=True,
                            )
                        if gh == gph - 1:
                            nc.scalar.copy(attn_sb[:], psat_hs[g // gph][:])
                            boundary(l, attn_sb, g // gph)

                # ================= layer 2 =================
                for g in range(groups):
                    npg = np_g(g)
                    if compute_only:
                        if g == 0:
                            kt2_c = singles.tile([128, 16, npad0], FP8)
                            nc.sync.dma_start(kt2_c[:], k20[:])
                        kt2 = kt2_c
                    elif g == 0:
                        kt2 = k2pool.tile([128, 16, npad0], FP8, tag="kt20",
                                          bufs=1)
                        nc.sync.dma_start(kt2[:], k20[:])
                    else:
                        kt2 = k2pool.tile([128, 16, npadr], FP8, tag="kt2r")
                        nc.sync.dma_start(kt2[:], k2r[g - 1])
                    ps_s2 = p_s.tile([128, npad0], F32, tag="ps_s")
                    for e in range(DC):
                        for k in range(4):
                            b = 4 * g + k
                            nc.tensor.matmul(
                                ps_s2[32 * k:32 * k + 32, :npg],
                                qf_pad[:, e, 32 * b:32 * b + 32],
                                kt2[:, 4 * k + e, :],
                                start=(e == 0),
                                stop=(e == DC - 1),
                                tile_position=(0, 32 * k),
                            )
                    u_t = work.tile([128, npad0], F32, tag="u_t")
                    nc.scalar.activation(
                        u_t[:, :npg], ps_s2[:, :npg],
                        mybir.ActivationFunctionType.Tanh,
                        scale=CT,
                    )
                    zsum2 = small.tile([128, 1], F32, tag="zsum2")
                    e2_t = work.tile([128, npad0], F32, tag="e2_t")
                    nc.scalar.activation(
                        e2_t[:, :npg], u_t[:, :npg],
                        mybir.ActivationFunctionType.Exp,
                        bias=sb_nclip[:], scale=CLIP, accum_out=zsum2[:],
                    )
                    zc2 = small.tile([128, 1], F32, tag="zc2")
                    nc.vector.tensor_scalar_add(zc2[:], zsum2[:],
                                                sb_nzc2[:, g:g + 1])
                    rz2 = small.tile([128, 1], F32, tag="rz2")
                    nc.vector.reciprocal(rz2[:], zc2[:])
                    w2_t = work.tile([128, npad0], F32, tag="w2_t")
                    nc.vector.tensor_scalar_mul(w2_t[:, :npg], e2_t[:, :npg],
                                                rz2[:])
                    nc.sync.dma_start(
                        out[4 * g:4 * g + 4, :npg],
                        w2_t.rearrange("(k r) n -> k r n", r=32)[:, 0, :npg],
                    )

            def _emit_dma_only():
                """Same DMA stream as the real kernel; tiny DVE consumers keep
                each tile live. Measures the DMA floor."""
                sink = small.tile([128, 1], F32, tag="sink")
                for l in range(2):
                    for g in range(groups):
                        if g == 0:
                            kt = kvpool.tile([128, 16, npad0], FP8, tag="kt0",
                                             bufs=2)
                            nc.sync.dma_start(kt[:], kt0[l])
                            v = kvpool.tile([128, 4 * nfull0, D], FP8,
                                            tag="v0", bufs=2)
                            nc.sync.dma_start(v[:], v0[l])
                        else:
                            kt = kvpool.tile([128, 16, npadr], FP8, tag="ktr")
                            nc.sync.dma_start(kt[:],
                                              ktr[l * (groups - 1) + g - 1])
                            v = kvpool.tile([128, 4 * nfullr, D], FP8,
                                            tag="vr")
                            nc.sync.dma_start(v[:],
                                              vr[l * (groups - 1) + g - 1])
                        nc.vector.tensor_reduce(
                            sink[:], kt[:, 0, :4], axis=mybir.AxisListType.X,
                            op=mybir.AluOpType.add,
                        )
                        nc.vector.tensor_reduce(
                            sink[:], v[:, 0, :4], axis=mybir.AxisListType.X,
                            op=mybir.AluOpType.add,
                        )
                w2_t = work.tile([128, npad0], F32, tag="w2_t")
                nc.vector.memset(w2_t[:], 0.5)
                for g in range(groups):
                    if g == 0:
                        kt2 = k2pool.tile([128, 16, npad0], FP8, tag="kt20",
                                          bufs=1)
                        nc.sync.dma_start(kt2[:], k20[:])
                    else:
                        kt2 = k2pool.tile([128, 16, npadr], FP8, tag="kt2r")
                        nc.sync.dma_start(kt2[:], k2r[g - 1])
                    nc.vector.tensor_reduce(
                        sink[:], kt2[:, 0, :4], axis=mybir.AxisListType.X,
                        op=mybir.AluOpType.add,
                    )
                    nc.sync.dma_start(
                        out[4 * g:4 * g + 4, :],
                        w2_t.rearrange("(k r) n -> k r n", r=32)[:, 0, :],
                    )

            import contextlib
            if unroll and reps > 1:
                # Python-unrolled reps: no HW-loop barrier, so the Tile
                # scheduler pipelines across reps — steady-state throughput
                for r in range(reps):
                    if r:
                        nc.sync.dma_start(sb_qbd[:], qbd0[:])
                    if dma_only:
                        _emit_dma_only()
                    else:
                        _emit_body()
            else:
                loop_cm = (tc.For_i(0, reps, 1, staggered_reset=staggered)
                           if reps > 1 else contextlib.nullcontext())
                with loop_cm:
                    if reps > 1:
                        # re-load the block-diag query so each rep is identical
                        nc.sync.dma_start(sb_qbd[:], qbd0[:])
                    if dma_only:
                        _emit_dma_only()
                    else:
                        _emit_body()

    _hoist_excess_matmul_waits(nc)
    return nc


# ---------------- host-side preparation ----------------

def _host_constants():
    import ml_dtypes
    p = np.arange(128)
    # selY[p, d] = 1 iff (p % 32) == d // 32   (slot row p holds head p%32;
    # head h owns d in [32h, 32h+32); pad rows 16..31 never match)
    selY = ((p[:, None] % 32) == (np.arange(D)[None, :] // 32)).astype(np.float32)
    r = np.arange(4)
    slotsel = ((p[:, None] // 32) == r[None, :]).astype(ml_dtypes.bfloat16)
    ident = np.eye(128, dtype=np.float32)
    return selY, slotsel, ident


def _prep_weights(W0_w, W0_b, Wq_w, Wq_b):
    # attn_dev = SV*attn (w = e/Z exact f32, V at SV scale)
    s0 = SQ / SK
    import ml_dtypes
    w0t = (np.asarray(W0_w, np.float32).T * s0).reshape(DC, 128, D)
    w0t = np.ascontiguousarray(w0t.transpose(1, 0, 2)).astype(ml_dtypes.bfloat16)
    wqt = np.asarray(Wq_w, np.float32).T.reshape(DC, 128, D)
    wqt = np.ascontiguousarray(wqt.transpose(1, 0, 2)).astype(ml_dtypes.bfloat16)
    b0t = np.ascontiguousarray(
        (np.asarray(W0_b, np.float32) * SQ).reshape(DC, 128).T)
    bqt = np.ascontiguousarray(
        (np.asarray(Wq_b, np.float32) * SQ).reshape(DC, 128).T)
    return w0t, wqt, b0t, bqt


def _quant8(x):
    import ml_dtypes
    return np.asarray(x, np.float32).astype(ml_dtypes.float8_e4m3)


def global_perm(mask, npad0, npadr):
    """Deal batches so each core's group 0 holds its 4 largest legal-count
    batches.  Returns perm (kernel batch order: perm[i] = original batch
    index of kernel slot i) or None if the input does not fit."""
    cnt = (~mask).sum(axis=1)
    if cnt.max() > npad0:
        return None
    b_core = mask.shape[0] // N_CORES
    order = np.argsort(-cnt, kind="stable")
    # snake-deal the 8*4 largest into the cores' group-0 slots
    g0 = order[:4 * N_CORES].reshape(4, N_CORES)
    g0[1::2] = g0[1::2, ::-1]
    rest = np.sort(order[4 * N_CORES:])
    perm = np.empty(mask.shape[0], np.int64)
    for c in range(N_CORES):
        perm[c * b_core:c * b_core + 4] = g0[:, c]
        perm[c * b_core + 4:(c + 1) * b_core] = rest[c * (b_core - 4):
                                                     (c + 1) * (b_core - 4)]
    # every non-group-0 slot must fit in npadr
    if cnt[perm[np.arange(mask.shape[0]) % b_core >= 4]].max() > npadr:
        return None
    return perm


def _compact_idx(mask, caps):
    """Per-batch legal-rows-first gather index [b, max(caps)] + counts.
    caps[i] = that slot's capacity; pads repeat index 0 (K=V zeroed)."""
    b = mask.shape[0]
    mcap = int(max(caps))
    idx = np.zeros((b, mcap), np.int64)
    cnt = np.zeros((b,), np.int64)
    for i in range(b):
        legal = np.flatnonzero(~mask[i])
        assert len(legal) <= caps[i]
        cnt[i] = len(legal)
        idx[i, :len(legal)] = legal
    return idx, cnt


def _prep_kv_core(K_c, V_c, idx, cnt, b_core, npad0, npadr):
    """Build kt0/ktr, v0/vr, vt, k20/k2r (all fp8e4m3 at 16x scale) for one
    core's batch shard, rows compacted via idx; pad rows zeroed."""
    groups = b_core // 4
    nfull0 = npad0 // 128
    ntail0 = npad0 - 128 * nfull0
    nfullr = npadr // 128
    ar = np.arange(b_core)[:, None]
    mcap = idx.shape[1]
    pad = np.arange(mcap)[None, :] >= cnt[:, None]             # [b, mcap]
    Kg = np.asarray(K_c, np.float32)[ar, idx]                  # [b, mcap, 3D]
    Vg = np.asarray(V_c, np.float32)[ar, idx]
    Kg[pad] = 0.0
    Vg[pad] = 0.0
    K8 = _quant8(SK * Kg)
    V8 = _quant8(SK * Vg)

    def kt_of(K8s, npad):
        """[nb, npad, 3D] -> [3, nb//4, 128, 16, npad]"""
        nb = K8s.shape[0]
        Kv = K8s[:, :npad].reshape(nb // 4, 4, npad, 3, DC, 128)
        return np.ascontiguousarray(Kv.transpose(3, 0, 5, 1, 4, 2)).reshape(
            3, nb // 4, 128, 16, npad)

    def v_of(V8s, nfull):
        nb = V8s.shape[0]
        Vv = V8s[:, :128 * nfull].reshape(nb // 4, 4, nfull, 128, 3, D)
        return np.ascontiguousarray(Vv.transpose(4, 0, 3, 1, 2, 5)).reshape(
            3, nb // 4, 128, 4 * nfull, D)

    kt0_3 = kt_of(K8[:4], npad0)           # [3, 1, 128, 16, npad0]
    ktr_3 = kt_of(K8[4:], npadr)           # [3, G-1, 128, 16, npadr]
    v0_3 = v_of(V8[:4], nfull0)
    vr_3 = v_of(V8[4:], nfullr)
    kt0 = np.ascontiguousarray(kt0_3[:2, 0])                   # [2, 128, 16, npad0]
    ktr = np.ascontiguousarray(ktr_3[:2]).reshape(2 * (groups - 1), 128, 16,
                                                  npadr)
    v0 = np.ascontiguousarray(v0_3[:2, 0])
    vr = np.ascontiguousarray(vr_3[:2]).reshape(2 * (groups - 1), 128,
                                                4 * nfullr, D)
    k20 = np.ascontiguousarray(kt0_3[2, 0])
    k2r = np.ascontiguousarray(ktr_3[2])
    vt = None
    if ntail0:
        # vt[r, l, k, d] = V8[k, 128*nfull0 + r, l*D + d]
        Vt = V8[:4, 128 * nfull0:npad0].reshape(4, ntail0, 3, D)
        vt = np.ascontiguousarray(Vt.transpose(1, 2, 0, 3)[:, :2])
    return kt0, ktr, v0, vr, vt, k20, k2r


def _prep_core(query_c, cnt, b_core, npad0, npadr):
    """Per-core fp8 block-diag query + f32 Z-correction tensors."""
    groups = b_core // 4
    qs = SK * np.asarray(query_c[:, 0, :], np.float32)          # [b, D]
    qbd = np.zeros((128, DC, b_core, 32), np.float32)
    for e in range(DC):
        for g in range(4):
            # rows 32g..32g+32 of chunk e hold d = 128e + 32g .., head 4e+g
            qbd[32 * g:32 * g + 32, e, :, 4 * e + g] = qs[:, 128 * e + 32 * g:
                                                          128 * e + 32 * g + 32].T
    qbd8 = _quant8(qbd.reshape(128, DC, b_core * 32))
    # nzc[p, g] = -(npad_g - cnt[4g + p//32]); nzc2 scales by exp(-CLIP)
    caps = np.where(np.arange(b_core) < 4, npad0, npadr)
    padn = (caps - cnt).astype(np.float32).reshape(groups, 4)  # [g, k]
    nzc = -np.ascontiguousarray(
        np.repeat(padn, 32, axis=1).T).astype(np.float32)      # [128, g]
    nzc2 = np.float32(np.exp(-CLIP)) * nzc
    return qbd8, nzc, nzc2


def prep_in_maps(query, K_att, V_att, mask, W0_w, W0_b, Wq_w, Wq_b,
                 npad0, npadr, perm):
    """Build the 8 per-core input maps + the output scatter indices.
    perm is the kernel-order -> original-batch mapping (identity allowed)."""
    query = np.asarray(query, np.float32)[perm]
    K_att = np.asarray(K_att, np.float32)[perm]
    V_att = np.asarray(V_att, np.float32)[perm]
    mask = np.asarray(mask).astype(bool)[perm]
    b_core = B // N_CORES
    selY, slotsel, ident = _host_constants()
    w0t, wqt, b0t, bqt = _prep_weights(W0_w, W0_b, Wq_w, Wq_b)
    caps = np.where(np.arange(B) % b_core < 4, npad0, npadr)
    idx_all, cnt_all = _compact_idx(mask, caps)
    in_maps = []
    for i in range(N_CORES):
        sl = slice(i * b_core, (i + 1) * b_core)
        idx, cnt = idx_all[sl], cnt_all[sl]
        qbd8, nzc, nzc2 = _prep_core(query[sl], cnt, b_core, npad0, npadr)
        kt0, ktr, v0, vr, vt, k20, k2r = _prep_kv_core(
            K_att[sl], V_att[sl], idx, cnt, b_core, npad0, npadr)
        m = {
            "kt0": kt0, "ktr": ktr, "v0": v0, "vr": vr,
            "k20": k20, "k2r": k2r, "qbd0": qbd8,
            "w0t": w0t, "wqt": wqt, "b0t": b0t, "bqt": bqt,
            "nzc": nzc, "nzc2": nzc2,
            "slotsel": slotsel, "sely": selY, "ident": ident,
        }
        if vt is not None:
            m["vt"] = vt
        in_maps.append(m)
    return in_maps, idx_all, cnt_all


def scatter_out(res_out, idx_all, cnt_all, perm):
    """[B, npad0] compacted (permuted) weights -> [B, N] full output."""
    npad = res_out.shape[1]
    full = np.zeros((B, N + 1), np.float32)
    j = np.arange(npad)[None, :]
    tgt = np.where(j < cnt_all[:, None], idx_all[:, :npad], N)
    np.put_along_axis(full, tgt, np.asarray(res_out, np.float32), axis=1)
    unperm = np.empty(B, np.int64)
    unperm[perm] = np.arange(B)
    return full[unperm, :N]


_NC_CACHE = {}
TRACE = False          # test-harness hook: profile the run, fill LAST
LAST = {}


def plan_shapes(mask):
    """Choose capacities + batch permutation for this input."""
    mask = np.asarray(mask).astype(bool)
    perm = global_perm(mask, NPAD0, NPADR)
    if perm is not None:
        return NPAD0, NPADR, perm
    return N, N, np.arange(B)      # uncompacted fallback, always correct


def kernel(query, K_att, V_att, mask, W0_w, W0_b, Wq_w, Wq_b):
    from concourse.bass_utils import run_bass_kernel_spmd

    b_core = B // N_CORES
    npad0, npadr, perm = plan_shapes(mask)

    if (b_core, npad0, npadr) not in _NC_CACHE:
        _NC_CACHE[(b_core, npad0, npadr)] = build_nc(b_core, npad0=npad0,
                                                     npadr=npadr)
    nc = _NC_CACHE[(b_core, npad0, npadr)]

    in_maps, idx_all, cnt_all = prep_in_maps(
        query, K_att, V_att, mask, W0_w, W0_b, Wq_w, Wq_b,
        npad0, npadr, perm)

    rr = run_bass_kernel_spmd(nc, in_maps, list(range(N_CORES)), trace=TRACE)
    LAST["exec_time_ns"] = rr.exec_time_ns
    res = rr.results
    out_c = np.concatenate([res[i]["out"] for i in range(N_CORES)], axis=0)
    return scatter_out(out_c, idx_all, cnt_all, perm)


# revision 28
# speedup vs baseline: 1.0854x; 1.0854x over previous
"""Trainium2 Bass kernel for nn_CPDP_AM_net_SGBS (3-layer MHA decoder step), v4.

Contract: kernel(**inputs) takes FULL inputs (B=256) and returns the FULL
output (256, 512).  Internally shards the batch dim across 8 NeuronCores
(32 batches/core), data-parallel, no cross-core communication.

v4 strategy (memory-regime), building on v2 (fp8 host quantization,
host-pre-transposed K, block-diag query scores, y-form AV) and v3
(mask-based row compaction):

  - The ~30% of K/V rows the mask forbids are dropped host-side: each
    batch's legal rows are gathered front-first.  A global batch
    permutation (undone in the host-side scatter) deals the largest-count
    batches round-robin across cores and into group 0 of each core, so
    GROUP 0 runs with capacity NPAD0=416 (3 full 128-row n-chunks + one
    32-row V-tail matmul) while GROUPS 1..7 run with capacity NPADR=384
    (3 full chunks, no tail).  Seed-0 inputs: max legal rows 388, five
    batches exceed 384 -> easily placed.  Any input that does not fit
    falls back to uniform npad 512 (same code path, always correct).
  - Pad slots carry K=V=0, so their scores are exactly 0 and their
    exp contributes exactly 1.0 (layers 0/1) / exp(-CLIP) (layer 2) to
    the softmax denominator; the exact per-batch count is subtracted
    on-device (nzc/nzc2 via DVE) — no -1e9 bias matmuls at all.
  - softmax without max-subtraction (logits are tiny): ACT exp(scale=cs)
    with fused row-sum, DVE reciprocal, 1/Z deferred into the head-diag
    extraction (scalar_tensor_tensor with selY).
  - w'^T for AV via PE transposes + ACT fp8 cast (default) or DVE
    square-block transposes in bf16 (dve_t=True).
  - head-diag slot-row sums as 4 tiny PE matmuls vs a 0/1 selector.
  - deep KV/K2 prefetch (kv_bufs=7, k2_bufs=6: layer-2 K streams during
    layer-1 compute) and PSUM rebalanced to 3 score banks (ps_bufs=3,
    pz_bufs=1) — together -6% sustained on HW.
  - projections (W0, Wq) as accumulated matmuls with host-scaled
    transposed bf16 weights, emitted per half (halves=2) so they overlap
    the second half of each layer's groups.

HBM traffic per core: ~31.9 MB (vs 42 MB v2, 168 MB plain fp32).

Scale ledger (host <-> device):
  K8 = e4(16*K), V8 = e4(16*V), q0_8 = e4(16*q0), w8 = e4(exp(logit)) ~ 1
  (unnormalized; 1/Z folded into the extraction via scalar_tensor_tensor)
  attn_dev = 16*attn ; W0h = (100/16)*W0^T -> q1_dev = 100*q1 (same for
  q2); Wqh = Wq^T -> qf_dev = 100*qf.
  exp scales: cs0 = 1/(16*16*sqrt(32)), cs1 = 1/(100*16*sqrt(32));
  tanh scale: ct = 1/(100*16*sqrt(512)).
"""

import sys

if "/opt/trn_rl_repo" not in sys.path:
    sys.path.insert(0, "/opt/trn_rl_repo")

import numpy as np

import concourse.bass as bass
import concourse.tile as tile
import concourse.mybir as mybir

F32 = mybir.dt.float32
BF16 = mybir.dt.bfloat16
FP8 = mybir.dt.float8e4

N_CORES = 8
B = 256
N = 512
D = 512
H = 16
DH = 32
DC = 4                # d chunks of 128
NPAD0 = 416           # group-0 capacity (13*32)
NPADR = 384           # groups 1+ capacity (3*128)
CLIP = 10.0

SK = 16.0             # fp8 scale for K, V, q0
SQ = 100.0            # device scale of q1/q2/qf
CS0 = 1.0 / (SK * SK * np.sqrt(DH))
CS1 = 1.0 / (SQ * SK * np.sqrt(DH))
CT = 1.0 / (SQ * SK * np.sqrt(D))


def _hoist_excess_matmul_waits(nc, keep=1):
    """walrus limits self-loading 4-byte matmuls (fp32/fp32r/transpose) to a
    single sync wait on the S3_LW struct.  Hoist excess waits onto a
    standalone PE EventSemaphore inserted right before the matmul — same
    engine, so per-engine program order makes it equivalent."""
    for fn in nc.m.functions:
        for blk in fn.blocks:
            il = blk.instructions
            i = 0
            while i < len(il):
                inst = il[i]
                si = inst.sync_info
                if (type(inst).__name__ != "InstEventSemaphore"
                        and si is not None
                        and si.on_wait and len(si.on_wait) > keep):
                    moved = list(si.on_wait[:-keep]) if keep else list(si.on_wait)
                    kept = list(si.on_wait[-keep:]) if keep else []
                    for j, w in enumerate(moved):
                        wi = mybir.InstEventSemaphore(
                            name=f"{inst.name}-hw{j}",
                            ins=[], outs=[],
                            sync_info=mybir.SyncInfo(on_wait=[w], on_update=[]),
                        )
                        wi.engine = inst.engine
                        nc.register_instruction(wi)
                        il.insert(i, wi)
                        i += 1
                    inst.sync_info = mybir.SyncInfo(
                        on_wait=kept, on_update=list(si.on_update)
                    )
                i += 1


def build_nc(b_core=32, npad0=NPAD0, npadr=NPADR, reps=1, dma_only=False,
             kv_bufs=7, k2_bufs=6, compute_only=False, staggered=False,
             unroll=False, dve_t=False, halves=2, ps_bufs=3, pz_bufs=1):
    """Build the single-core Bass program for a [b_core]-batch shard with
    group-0 n-capacity npad0 (tail = npad0 % 128 via a resident V-tail
    tile) and npadr for the remaining groups (npadr % 128 must be 0,
    or npadr == npad0)."""
    groups = b_core // 4
    nfull0 = npad0 // 128
    ntail0 = npad0 - 128 * nfull0     # 0 or a multiple of 32
    nfullr = npadr // 128
    assert npadr == npad0 or npadr % 128 == 0
    assert nfull0 == nfullr or ntail0 == 0
    nc = bass.Bass()

    def np_g(g):
        return npad0 if g == 0 else npadr

    # K^T chunks (idx 4k+e: partition p = d within chunk e of batch k,
    # free = compacted n); group 0 vs rest have different n capacity
    kt0 = nc.declare_dram_parameter("kt0", [2, 128, 16, npad0], FP8,
                                    isOutput=False)
    ktr = nc.declare_dram_parameter("ktr", [2 * (groups - 1), 128, 16, npadr],
                                    FP8, isOutput=False)
    # V full chunks (idx k*nfull+c: partition p = n within chunk c of
    # batch k, free = d); same shape for both classes (nfull equal)
    v0 = nc.declare_dram_parameter("v0", [2, 128, 4 * nfull0, D], FP8,
                                   isOutput=False)
    vr = nc.declare_dram_parameter("vr", [2 * (groups - 1), 128, 4 * nfullr, D],
                                   FP8, isOutput=False)
    if ntail0:
        vt = nc.declare_dram_parameter("vt", [ntail0, 2, 4, D], FP8,
                                       isOutput=False)
    k20 = nc.declare_dram_parameter("k20", [128, 16, npad0], FP8,
                                    isOutput=False)
    k2r = nc.declare_dram_parameter("k2r", [groups - 1, 128, 16, npadr], FP8,
                                    isOutput=False)
    qbd0 = nc.declare_dram_parameter("qbd0", [128, DC, b_core * 32], FP8,
                                     isOutput=False)
    w0t = nc.declare_dram_parameter("w0t", [128, DC, D], BF16, isOutput=False)
    wqt = nc.declare_dram_parameter("wqt", [128, DC, D], BF16, isOutput=False)
    b0t = nc.declare_dram_parameter("b0t", [128, DC], F32, isOutput=False)
    bqt = nc.declare_dram_parameter("bqt", [128, DC], F32, isOutput=False)
    nzc = nc.declare_dram_parameter("nzc", [128, groups], F32, isOutput=False)
    nzc2 = nc.declare_dram_parameter("nzc2", [128, groups], F32, isOutput=False)
    slotsel = nc.declare_dram_parameter("slotsel", [128, 4], BF16, isOutput=False)
    sely = nc.declare_dram_parameter("sely", [128, D], F32, isOutput=False)
    ident = nc.declare_dram_parameter("ident", [128, 128], F32, isOutput=False)
    out = nc.declare_dram_parameter("out", [b_core, npad0], F32, isOutput=True)

    with tile.TileContext(nc) as tc:
        with (
            tc.tile_pool(name="singles", bufs=1) as singles,
            tc.tile_pool(name="kvpool", bufs=kv_bufs) as kvpool,
            tc.tile_pool(name="k2pool", bufs=k2_bufs) as k2pool,
            tc.tile_pool(name="work", bufs=3) as work,
            tc.tile_pool(name="small", bufs=8) as small,
            tc.tile_pool(name="p_s", bufs=ps_bufs, space="PSUM") as p_s,
            tc.tile_pool(name="p_y", bufs=2, space="PSUM") as p_y,
            tc.tile_pool(name="p_wt", bufs=1, space="PSUM") as p_wt,
            tc.tile_pool(name="p_zt", bufs=pz_bufs, space="PSUM") as p_zt,
            tc.tile_pool(name="p_q", bufs=1, space="PSUM") as p_q,
        ):
            # ---- constants / weights ----
            sb_qbd = singles.tile([128, DC, b_core * 32], FP8)
            nc.sync.dma_start(sb_qbd[:], qbd0[:])
            if ntail0:
                sb_vt = singles.tile([ntail0, 2, 4, D], FP8)
                nc.sync.dma_start(sb_vt[:], vt[:])
            sb_w0t = singles.tile([128, DC, D], BF16)
            nc.sync.dma_start(sb_w0t[:], w0t[:])
            sb_wqt = singles.tile([128, DC, D], BF16)
            nc.sync.dma_start(sb_wqt[:], wqt[:])
            sb_b0t = singles.tile([128, DC], F32)
            nc.sync.dma_start(sb_b0t[:], b0t[:])
            sb_bqt = singles.tile([128, DC], F32)
            nc.sync.dma_start(sb_bqt[:], bqt[:])
            sb_nzc = singles.tile([128, groups], F32)
            nc.sync.dma_start(sb_nzc[:], nzc[:])
            sb_nzc2 = singles.tile([128, groups], F32)
            nc.sync.dma_start(sb_nzc2[:], nzc2[:])
            sb_slotsel = singles.tile([128, 4], BF16)
            nc.sync.dma_start(sb_slotsel[:], slotsel[:])
            sb_selyf = singles.tile([128, D], F32)
            nc.sync.dma_start(sb_selyf[:], sely[:])
            sb_ident = singles.tile([128, 128], F32)
            nc.sync.dma_start(sb_ident[:], ident[:])
            sb_nclip = singles.tile([128, 1], F32)
            nc.vector.memset(sb_nclip[:], -CLIP)
            sb_identb = singles.tile([128, 128], BF16)
            nc.vector.tensor_copy(sb_identb[:], sb_ident[:])

            # qf_pad zeroing is off the critical path: do it first (once —
            # unrolled reps rewrite the same lanes each trip)
            qf_pad = singles.tile([128, DC, b_core * 32], FP8)
            nc.vector.memset(qf_pad[:], 0.0)
            qf_v = qf_pad.rearrange("p e (b j) -> p e b j", j=32)

            def softmax_e(ps_s, cs, g, npg):
                """psum scores [128, npg] -> unnormalized e [128, npg] SBUF
                + rz = 1/sum(e) over legal slots; pad slots contribute
                exp(0)=1 each, subtracted exactly via nzc."""
                zsum = small.tile([128, 1], F32, tag="zsum")
                e_t = work.tile([128, npad0], BF16, tag="e_t")
                nc.scalar.activation(
                    e_t[:, :npg], ps_s[:, :npg],
                    mybir.ActivationFunctionType.Exp,
                    scale=cs, accum_out=zsum[:],
                )
                zc = small.tile([128, 1], F32, tag="zc")
                nc.vector.tensor_scalar_add(zc[:], zsum[:], sb_nzc[:, g:g + 1])
                rz = small.tile([128, 1], F32, tag="rz")
                nc.vector.reciprocal(rz[:], zc[:])
                return e_t, rz

            def projection(attn_h, wt, bt, tag, hb, out_dt=F32):
                """q_nextT [128, DC(j), hb] = W^T @ attn^T + bias, for a
                half-layer batch slice (hb batches)."""
                ps_q = p_q.tile([128, DC, hb], F32, tag="ps_q")
                for jc in range(DC):
                    for ic in range(DC):
                        nc.tensor.matmul(
                            ps_q[:, jc, :],
                            wt[:, ic, 128 * jc:128 * jc + 128],
                            attn_h[:, ic, :],
                            start=(ic == 0), stop=(ic == DC - 1),
                        )
                qt = work.tile([128, DC, hb], out_dt, tag=tag)
                for jc in range(DC):
                    nc.vector.tensor_scalar_add(
                        qt[:, jc, :], ps_q[:, jc, :], bt[:, jc:jc + 1]
                    )
                return qt

            def fill_qbd_half(qt, h, hb):
                """Overwrite the block-diagonal of sb_qbd for batches
                [h*hb, (h+1)*hb) from qt [128, DC, hb] (f32 -> fp8 cast;
                split across DVE and ACT)."""
                qbd_v = sb_qbd.rearrange("p e (b j) -> p e b j", j=32)
                for e in range(DC):
                    for g in range(4):
                        dst = qbd_v[32 * g:32 * g + 32, e,
                                    h * hb:(h + 1) * hb, 4 * e + g]
                        src = qt[32 * g:32 * g + 32, e, :]
                        if (e + g) % 2 == 0:
                            nc.vector.tensor_copy(dst, src)
                        else:
                            nc.scalar.copy(dst, src)

            def _emit_body():
                gph = groups // halves          # groups per half
                hb = b_core // halves           # batches per half

                def boundary(l, attn_h, h):
                    """Projection + query-constant fill for half h of layer l;
                    emitted as soon as that half's groups are done."""
                    if l == 0:
                        qt1 = projection(attn_h, sb_w0t, sb_b0t, "qt1", hb)
                        fill_qbd_half(qt1, h, hb)
                    else:
                        q2t = projection(attn_h, sb_w0t, sb_b0t, "qt2", hb,
                                         out_dt=BF16)
                        qft = projection(q2t, sb_wqt, sb_bqt, "qft", hb)
                        for e in range(DC):
                            nc.vector.tensor_copy(
                                qf_v[:, e, h * hb:(h + 1) * hb, 0],
                                qft[:, e, :],
                            )

                # ================= layers 0, 1 =================
                for l in range(2):
                    cs = CS0 if l == 0 else CS1
                    attn_hs = [work.tile([128, DC, hb], BF16, tag=f"attn{h}",
                                         name=f"attn{h}")
                               for h in range(halves)]
                    psat_hs = [p_zt.tile([128, DC, hb], F32, tag="ps_at",
                                         name=f"psat{h}")
                               for h in range(halves)]
                    for g in range(groups):
                        npg = np_g(g)
                        nfg = nfull0 if g == 0 else nfullr
                        ntg = ntail0 if g == 0 else 0
                        attn_sb = attn_hs[g // gph]
                        gh = g % gph            # group index within the half
                        if compute_only:
                            if g == 0 and l == 0:
                                kt_c = singles.tile([128, 16, npad0], FP8)
                                nc.sync.dma_start(kt_c[:], kt0[0])
                                v_c = singles.tile([128, 4 * nfull0, D], FP8)
                                nc.sync.dma_start(v_c[:], v0[0])
                            kt, v = kt_c, v_c
                        elif g == 0:
                            kt = kvpool.tile([128, 16, npad0], FP8, tag="kt0",
                                             bufs=2)
                            nc.sync.dma_start(kt[:], kt0[l])
                            v = kvpool.tile([128, 4 * nfull0, D], FP8,
                                            tag="v0", bufs=2)
                            nc.sync.dma_start(v[:], v0[l])
                        else:
                            kt = kvpool.tile([128, 16, npadr], FP8, tag="ktr")
                            nc.sync.dma_start(kt[:],
                                              ktr[l * (groups - 1) + g - 1])
                            v = kvpool.tile([128, 4 * nfullr, D], FP8,
                                            tag="vr")
                            nc.sync.dma_start(v[:],
                                              vr[l * (groups - 1) + g - 1])
                        # scores: 4 batches col-tiled, wave-major over e
                        ps_s = p_s.tile([128, npad0], F32, tag="ps_s")
                        for e in range(DC):
                            for k in range(4):
                                b = 4 * g + k
                                nc.tensor.matmul(
                                    ps_s[32 * k:32 * k + 32, :npg],
                                    sb_qbd[:, e, 32 * b:32 * b + 32],
                                    kt[:, 4 * k + e, :],
                                    start=(e == 0),
                                    stop=(e == DC - 1),
                                    tile_position=(0, 32 * k),
                                )
                        e_t, rz = softmax_e(ps_s, cs, g, npg)
                        # w'^T: PE transposes + ACT fp8 cast, or DVE
                        # square-block transposes (bf16)
                        if dve_t:
                            wt8 = work.tile([128, 4, 128], BF16, tag="wt8")
                            for c in range(nfg):
                                nc.vector.transpose(
                                    wt8[:, c, :], e_t[:, 128 * c:128 * c + 128]
                                )
                            for t in range(ntg // 32):
                                for k in range(4):
                                    nc.vector.transpose(
                                        wt8[32 * t:32 * t + 32, nfg,
                                            32 * k:32 * k + 32],
                                        e_t[32 * k:32 * k + 32,
                                            128 * nfg + 32 * t:
                                            128 * nfg + 32 * t + 32],
                                    )
                        else:
                            pwt = p_wt.tile([128, 4, 128], BF16, tag="pwt")
                            for c in range(nfg):
                                nc.tensor.transpose(
                                    pwt[:, c, :], e_t[:, 128 * c:128 * c + 128],
                                    sb_identb[:]
                                )
                            if ntg:
                                nc.tensor.transpose(
                                    pwt[:ntg, nfg, :],
                                    e_t[:, 128 * nfg:npg],
                                    sb_identb[:]
                                )
                            wt8 = work.tile([128, 4, 128], FP8, tag="wt8")
                            nc.scalar.copy(wt8[:, :nfg, :], pwt[:, :nfg, :])
                            if ntg:
                                nc.scalar.copy(wt8[:ntg, nfg, :],
                                               pwt[:ntg, nfg, :])
                        # AV y-form: 4 batches col-tiled, wave-major over c
                        ps_y = p_y.tile([128, D], F32, tag="ps_y")
                        for c in range(nfg):
                            for k in range(4):
                                nc.tensor.matmul(
                                    ps_y[32 * k:32 * k + 32, :],
                                    wt8[:, c, 32 * k:32 * k + 32],
                                    v[:, k * nfg + c, :],
                                    start=(c == 0),
                                    stop=(ntg == 0 and c == nfg - 1),
                                    tile_position=(0, 32 * k),
                                )
                        if ntg:
                            for k in range(4):
                                nc.tensor.matmul(
                                    ps_y[32 * k:32 * k + 32, :],
                                    wt8[:ntg, nfg, 32 * k:32 * k + 32],
                                    sb_vt[:, l, k, :],
                                    start=False, stop=True,
                                    tile_position=(0, 32 * k),
                                )
                        # head-diag extraction with fused 1/Z normalization:
                        # zt = (Y * rz) * selY  -> attn^T[:, :, b]
                        zt = work.tile([128, D], BF16, tag="zt")
                        nc.vector.scalar_tensor_tensor(
                            zt[:], ps_y[:], rz[:], sb_selyf[:],
                            op0=mybir.AluOpType.mult, op1=mybir.AluOpType.mult,
                        )
                        # slot-row sums as 4 tiny PE matmuls: zt chunk as
                        # stationary, 0/1 slot selector streaming ->
                        # attn^T[:, c, 4 batches] straight into PSUM
                        ps_at = psat_hs[g // gph]
                        for c in range(DC):
                            nc.tensor.matmul(
                                ps_at[:, c, 4 * gh:4 * gh + 4],
                                zt[:, 128 * c:128 * c + 128],
                                sb_slotsel[:],
                                start=True, stop=True,
                            )
                        if gh == gph - 1:
                            nc.scalar.copy(attn_sb[:], psat_hs[g // gph][:])
                            boundary(l, attn_sb, g // gph)

                # ================= layer 2 =================
                for g in range(groups):
                    npg = np_g(g)
                    if compute_only:
                        if g == 0:
                            kt2_c = singles.tile([128, 16, npad0], FP8)
                            nc.sync.dma_start(kt2_c[:], k20[:])
                        kt2 = kt2_c
                    elif g == 0:
                        kt2 = k2pool.tile([128, 16, npad0], FP8, tag="kt20",
                                          bufs=1)
                        nc.sync.dma_start(kt2[:], k20[:])
                    else:
                        kt2 = k2pool.tile([128, 16, npadr], FP8, tag="kt2r")
                        nc.sync.dma_start(kt2[:], k2r[g - 1])
                    ps_s2 = p_s.tile([128, npad0], F32, tag="ps_s")
                    for e in range(DC):
                        for k in range(4):
                            b = 4 * g + k
                            nc.tensor.matmul(
                                ps_s2[32 * k:32 * k + 32, :npg],
                                qf_pad[:, e, 32 * b:32 * b + 32],
                                kt2[:, 4 * k + e, :],
                                start=(e == 0),
                                stop=(e == DC - 1),
                                tile_position=(0, 32 * k),
                            )
                    u_t = work.tile([128, npad0], F32, tag="u_t")
                    nc.scalar.activation(
                        u_t[:, :npg], ps_s2[:, :npg],
                        mybir.ActivationFunctionType.Tanh,
                        scale=CT,
                    )
                    zsum2 = small.tile([128, 1], F32, tag="zsum2")
                    e2_t = work.tile([128, npad0], F32, tag="e2_t")
                    nc.scalar.activation(
                        e2_t[:, :npg], u_t[:, :npg],
                        mybir.ActivationFunctionType.Exp,
                        bias=sb_nclip[:], scale=CLIP, accum_out=zsum2[:],
                    )
                    zc2 = small.tile([128, 1], F32, tag="zc2")
                    nc.vector.tensor_scalar_add(zc2[:], zsum2[:],
                                                sb_nzc2[:, g:g + 1])
                    rz2 = small.tile([128, 1], F32, tag="rz2")
                    nc.vector.reciprocal(rz2[:], zc2[:])
                    w2_t = work.tile([128, npad0], F32, tag="w2_t")
                    nc.vector.tensor_scalar_mul(w2_t[:, :npg], e2_t[:, :npg],
                                                rz2[:])
                    nc.sync.dma_start(
                        out[4 * g:4 * g + 4, :npg],
                        w2_t.rearrange("(k r) n -> k r n", r=32)[:, 0, :npg],
                    )

            def _emit_dma_only():
                """Same DMA stream as the real kernel; tiny DVE consumers keep
                each tile live. Measures the DMA floor."""
                sink = small.tile([128, 1], F32, tag="sink")
                for l in range(2):
                    for g in range(groups):
                        if g == 0:
                            kt = kvpool.tile([128, 16, npad0], FP8, tag="kt0",
                                             bufs=2)
                            nc.sync.dma_start(kt[:], kt0[l])
                            v = kvpool.tile([128, 4 * nfull0, D], FP8,
                                            tag="v0", bufs=2)
                            nc.sync.dma_start(v[:], v0[l])
                        else:
                            kt = kvpool.tile([128, 16, npadr], FP8, tag="ktr")
                            nc.sync.dma_start(kt[:],
                                              ktr[l * (groups - 1) + g - 1])
                            v = kvpool.tile([128, 4 * nfullr, D], FP8,
                                            tag="vr")
                            nc.sync.dma_start(v[:],
                                              vr[l * (groups - 1) + g - 1])
                        nc.vector.tensor_reduce(
                            sink[:], kt[:, 0, :4], axis=mybir.AxisListType.X,
                            op=mybir.AluOpType.add,
                        )
                        nc.vector.tensor_reduce(
                            sink[:], v[:, 0, :4], axis=mybir.AxisListType.X,
                            op=mybir.AluOpType.add,
                        )
                w2_t = work.tile([128, npad0], F32, tag="w2_t")
                nc.vector.memset(w2_t[:], 0.5)
                for g in range(groups):
                    if g == 0:
                        kt2 = k2pool.tile([128, 16, npad0], FP8, tag="kt20",
                                          bufs=1)
                        nc.sync.dma_start(kt2[:], k20[:])
                    else:
                        kt2 = k2pool.tile([128, 16, npadr], FP8, tag="kt2r")
                        nc.sync.dma_start(kt2[:], k2r[g - 1])
                    nc.vector.tensor_reduce(
                        sink[:], kt2[:, 0, :4], axis=mybir.AxisListType.X,
                        op=mybir.AluOpType.add,
                    )
                    nc.sync.dma_start(
                        out[4 * g:4 * g + 4, :],
                        w2_t.rearrange("(k r) n -> k r n", r=32)[:, 0, :],
                    )

            import contextlib
            if unroll and reps > 1:
                # Python-unrolled reps: no HW-loop barrier, so the Tile
                # scheduler pipelines across reps — steady-state throughput
                for r in range(reps):
                    if r:
                        nc.sync.dma_start(sb_qbd[:], qbd0[:])
                    if dma_only:
                        _emit_dma_only()
                    else:
                        _emit_body()
            else:
                loop_cm = (tc.For_i(0, reps, 1, staggered_reset=staggered)
                           if reps > 1 else contextlib.nullcontext())
                with loop_cm:
                    if reps > 1:
                        # re-load the block-diag query so each rep is identical
                        nc.sync.dma_start(sb_qbd[:], qbd0[:])
                    if dma_only:
                        _emit_dma_only()
                    else:
                        _emit_body()

    _hoist_excess_matmul_waits(nc)
    return nc


# ---------------- host-side preparation ----------------

def _host_constants():
    import ml_dtypes
    p = np.arange(128)
    # selY[p, d] = 1 iff (p % 32) == d // 32   (slot row p holds head p%32;
    # head h owns d in [32h, 32h+32); pad rows 16..31 never match)
    selY = ((p[:, None] % 32) == (np.arange(D)[None, :] // 32)).astype(np.float32)
    r = np.arange(4)
    slotsel = ((p[:, None] // 32) == r[None, :]).astype(ml_dtypes.bfloat16)
    ident = np.eye(128, dtype=np.float32)
    return selY, slotsel, ident


def _prep_weights(W0_w, W0_b, Wq_w, Wq_b):
    # attn_dev = SV*attn (w = e/Z exact f32, V at SV scale)
    s0 = SQ / SK
    import ml_dtypes
    w0t = (np.asarray(W0_w, np.float32).T * s0).reshape(DC, 128, D)
    w0t = np.ascontiguousarray(w0t.transpose(1, 0, 2)).astype(ml_dtypes.bfloat16)
    wqt = np.asarray(Wq_w, np.float32).T.reshape(DC, 128, D)
    wqt = np.ascontiguousarray(wqt.transpose(1, 0, 2)).astype(ml_dtypes.bfloat16)
    b0t = np.ascontiguousarray(
        (np.asarray(W0_b, np.float32) * SQ).reshape(DC, 128).T)
    bqt = np.ascontiguousarray(
        (np.asarray(Wq_b, np.float32) * SQ).reshape(DC, 128).T)
    return w0t, wqt, b0t, bqt


def _quant8(x):
    import ml_dtypes
    return np.asarray(x, np.float32).astype(ml_dtypes.float8_e4m3)


def global_perm(mask, npad0, npadr):
    """Deal batches so each core's group 0 holds its 4 largest legal-count
    batches.  Returns perm (kernel batch order: perm[i] = original batch
    index of kernel slot i) or None if the input does not fit."""
    cnt = (~mask).sum(axis=1)
    if cnt.max() > npad0:
        return None
    b_core = mask.shape[0] // N_CORES
    order = np.argsort(-cnt, kind="stable")
    # snake-deal the 8*4 largest into the cores' group-0 slots
    g0 = order[:4 * N_CORES].reshape(4, N_CORES)
    g0[1::2] = g0[1::2, ::-1]
    rest = np.sort(order[4 * N_CORES:])
    perm = np.empty(mask.shape[0], np.int64)
    for c in range(N_CORES):
        perm[c * b_core:c * b_core + 4] = g0[:, c]
        perm[c * b_core + 4:(c + 1) * b_core] = rest[c * (b_core - 4):
                                                     (c + 1) * (b_core - 4)]
    # every non-group-0 slot must fit in npadr
    if cnt[perm[np.arange(mask.shape[0]) % b_core >= 4]].max() > npadr:
        return None
    return perm


def _compact_idx(mask, caps):
    """Per-batch legal-rows-first gather index [b, max(caps)] + counts.
    caps[i] = that slot's capacity; pads repeat index 0 (K=V zeroed)."""
    b = mask.shape[0]
    mcap = int(max(caps))
    idx = np.zeros((b, mcap), np.int64)
    cnt = np.zeros((b,), np.int64)
    for i in range(b):
        legal = np.flatnonzero(~mask[i])
        assert len(legal) <= caps[i]
        cnt[i] = len(legal)
        idx[i, :len(legal)] = legal
    return idx, cnt


def _prep_kv_core(K_c, V_c, idx, cnt, b_core, npad0, npadr):
    """Build kt0/ktr, v0/vr, vt, k20/k2r (all fp8e4m3 at 16x scale) for one
    core's batch shard, rows compacted via idx; pad rows zeroed."""
    groups = b_core // 4
    nfull0 = npad0 // 128
    ntail0 = npad0 - 128 * nfull0
    nfullr = npadr // 128
    ar = np.arange(b_core)[:, None]
    mcap = idx.shape[1]
    pad = np.arange(mcap)[None, :] >= cnt[:, None]             # [b, mcap]
    Kg = np.asarray(K_c, np.float32)[ar, idx]                  # [b, mcap, 3D]
    Vg = np.asarray(V_c, np.float32)[ar, idx]
    Kg[pad] = 0.0
    Vg[pad] = 0.0
    K8 = _quant8(SK * Kg)
    V8 = _quant8(SK * Vg)

    def kt_of(K8s, npad):
        """[nb, npad, 3D] -> [3, nb//4, 128, 16, npad]"""
        nb = K8s.shape[0]
        Kv = K8s[:, :npad].reshape(nb // 4, 4, npad, 3, DC, 128)
        return np.ascontiguousarray(Kv.transpose(3, 0, 5, 1, 4, 2)).reshape(
            3, nb // 4, 128, 16, npad)

    def v_of(V8s, nfull):
        nb = V8s.shape[0]
        Vv = V8s[:, :128 * nfull].reshape(nb // 4, 4, nfull, 128, 3, D)
        return np.ascontiguousarray(Vv.transpose(4, 0, 3, 1, 2, 5)).reshape(
            3, nb // 4, 128, 4 * nfull, D)

    kt0_3 = kt_of(K8[:4], npad0)           # [3, 1, 128, 16, npad0]
    ktr_3 = kt_of(K8[4:], npadr)           # [3, G-1, 128, 16, npadr]
    v0_3 = v_of(V8[:4], nfull0)
    vr_3 = v_of(V8[4:], nfullr)
    kt0 = np.ascontiguousarray(kt0_3[:2, 0])                   # [2, 128, 16, npad0]
    ktr = np.ascontiguousarray(ktr_3[:2]).reshape(2 * (groups - 1), 128, 16,
                                                  npadr)
    v0 = np.ascontiguousarray(v0_3[:2, 0])
    vr = np.ascontiguousarray(vr_3[:2]).reshape(2 * (groups - 1), 128,
                                                4 * nfullr, D)
    k20 = np.ascontiguousarray(kt0_3[2, 0])
    k2r = np.ascontiguousarray(ktr_3[2])
    vt = None
    if ntail0:
        # vt[r, l, k, d] = V8[k, 128*nfull0 + r, l*D + d]
        Vt = V8[:4, 128 * nfull0:npad0].reshape(4, ntail0, 3, D)
        vt = np.ascontiguousarray(Vt.transpose(1, 2, 0, 3)[:, :2])
    return kt0, ktr, v0, vr, vt, k20, k2r


def _prep_core(query_c, cnt, b_core, npad0, npadr):
    """Per-core fp8 block-diag query + f32 Z-correction tensors."""
    groups = b_core // 4
    qs = SK * np.asarray(query_c[:, 0, :], np.float32)          # [b, D]
    qbd = np.zeros((128, DC, b_core, 32), np.float32)
    for e in range(DC):
        for g in range(4):
            # rows 32g..32g+32 of chunk e hold d = 128e + 32g .., head 4e+g
            qbd[32 * g:32 * g + 32, e, :, 4 * e + g] = qs[:, 128 * e + 32 * g:
                                                          128 * e + 32 * g + 32].T
    qbd8 = _quant8(qbd.reshape(128, DC, b_core * 32))
    # nzc[p, g] = -(npad_g - cnt[4g + p//32]); nzc2 scales by exp(-CLIP)
    caps = np.where(np.arange(b_core) < 4, npad0, npadr)
    padn = (caps - cnt).astype(np.float32).reshape(groups, 4)  # [g, k]
    nzc = -np.ascontiguousarray(
        np.repeat(padn, 32, axis=1).T).astype(np.float32)      # [128, g]
    nzc2 = np.float32(np.exp(-CLIP)) * nzc
    return qbd8, nzc, nzc2


def prep_in_maps(query, K_att, V_att, mask, W0_w, W0_b, Wq_w, Wq_b,
                 npad0, npadr, perm):
    """Build the 8 per-core input maps + the output scatter indices.
    perm is the kernel-order -> original-batch mapping (identity allowed)."""
    query = np.asarray(query, np.float32)[perm]
    K_att = np.asarray(K_att, np.float32)[perm]
    V_att = np.asarray(V_att, np.float32)[perm]
    mask = np.asarray(mask).astype(bool)[perm]
    b_core = B // N_CORES
    selY, slotsel, ident = _host_constants()
    w0t, wqt, b0t, bqt = _prep_weights(W0_w, W0_b, Wq_w, Wq_b)
    caps = np.where(np.arange(B) % b_core < 4, npad0, npadr)
    idx_all, cnt_all = _compact_idx(mask, caps)
    in_maps = []
    for i in range(N_CORES):
        sl = slice(i * b_core, (i + 1) * b_core)
        idx, cnt = idx_all[sl], cnt_all[sl]
        qbd8, nzc, nzc2 = _prep_core(query[sl], cnt, b_core, npad0, npadr)
        kt0, ktr, v0, vr, vt, k20, k2r = _prep_kv_core(
            K_att[sl], V_att[sl], idx, cnt, b_core, npad0, npadr)
        m = {
            "kt0": kt0, "ktr": ktr, "v0": v0, "vr": vr,
            "k20": k20, "k2r": k2r, "qbd0": qbd8,
            "w0t": w0t, "wqt": wqt, "b0t": b0t, "bqt": bqt,
            "nzc": nzc, "nzc2": nzc2,
            "slotsel": slotsel, "sely": selY, "ident": ident,
        }
        if vt is not None:
            m["vt"] = vt
        in_maps.append(m)
    return in_maps, idx_all, cnt_all


def scatter_out(res_out, idx_all, cnt_all, perm):
    """[B, npad0] compacted (permuted) weights -> [B, N] full output."""
    npad = res_out.shape[1]
    full = np.zeros((B, N + 1), np.float32)
    j = np.arange(npad)[None, :]
    tgt = np.where(j < cnt_all[:, None], idx_all[:, :npad], N)
    np.put_along_axis(full, tgt, np.asarray(res_out, np.float32), axis=1)
    unperm = np.empty(B, np.int64)
    unperm[perm] = np.arange(B)
    return full[unperm, :N]


_NC_CACHE = {}
TRACE = False          # test-harness hook: profile the run, fill LAST
LAST = {}


def plan_shapes(mask):
    """Choose capacities + batch permutation for this input."""
    mask = np.asarray(mask).astype(bool)
    perm = global_perm(mask, NPAD0, NPADR)
    if perm is not None:
        return NPAD0, NPADR, perm
    return N, N, np.arange(B)      # uncompacted fallback, always correct


def kernel(query, K_att, V_att, mask, W0_w, W0_b, Wq_w, Wq_b):
    from concourse.bass_utils import run_bass_kernel_spmd

    b_core = B // N_CORES
    npad0, npadr, perm = plan_shapes(mask)

    if (b_core, npad0, npadr) not in _NC_CACHE:
        _NC_CACHE[(b_core, npad0, npadr)] = build_nc(b_core, npad0=npad0,
                                                     npadr=npadr)
    nc = _NC_CACHE[(b_core, npad0, npadr)]

    in_maps, idx_all, cnt_all = prep_in_maps(
        query, K_att, V_att, mask, W0_w, W0_b, Wq_w, Wq_b,
        npad0, npadr, perm)

    rr = run_bass_kernel_spmd(nc, in_maps, list(range(N_CORES)), trace=TRACE)
    LAST["exec_time_ns"] = rr.exec_time_ns
    res = rr.results
    out_c = np.concatenate([res[i]["out"] for i in range(N_CORES)], axis=0)
    return scatter_out(out_c, idx_all, cnt_all, perm)


# revision 33
# speedup vs baseline: 1.1009x; 1.0142x over previous
"""Trainium2 Bass kernel for nn_CPDP_AM_net_SGBS (3-layer MHA decoder step), v4.

Contract: kernel(**inputs) takes FULL inputs (B=256) and returns the FULL
output (256, 512).  Internally shards the batch dim across 8 NeuronCores
(32 batches/core), data-parallel, no cross-core communication.

v4 strategy (memory-regime), building on v2 (fp8 host quantization,
host-pre-transposed K, block-diag query scores, y-form AV) and v3
(mask-based row compaction):

  - The ~30% of K/V rows the mask forbids are dropped host-side: each
    batch's legal rows are gathered front-first.  A global batch
    permutation (undone in the host-side scatter) deals the largest-count
    batches round-robin across cores and into group 0 of each core, so
    GROUP 0 runs with capacity NPAD0=416 (3 full 128-row n-chunks + one
    32-row V-tail matmul) while GROUPS 1..7 run with capacity NPADR=384
    (3 full chunks, no tail).  Seed-0 inputs: max legal rows 388, five
    batches exceed 384 -> easily placed.  Any input that does not fit
    falls back to uniform npad 512 (same code path, always correct).
  - Pad slots carry K=V=0, so their scores are exactly 0 and their
    exp contributes exactly 1.0 (layers 0/1) / exp(-CLIP) (layer 2) to
    the softmax denominator; the exact per-batch count is subtracted
    on-device (nzc/nzc2 via DVE) — no -1e9 bias matmuls at all.
  - softmax without max-subtraction (logits are tiny): ACT exp(scale=cs)
    with fused row-sum, DVE reciprocal, 1/Z deferred into the head-diag
    extraction (scalar_tensor_tensor with selY).
  - w'^T for AV via PE transposes + fp8 cast on the DVE (keeps the ACT
    exp->cast dependency chain off the busier ACT engine).
  - head-diag slot-row sums as 4 tiny PE matmuls vs a 0/1 selector.
  - deep KV/K2 prefetch (kv_bufs=7, k2_bufs=6: layer-2 K streams during
    layer-1 compute) and PSUM rebalanced to 3 score banks (ps_bufs=3,
    pz_bufs=1) — together -6% sustained on HW.
  - projections (W0, Wq) as accumulated matmuls with host-scaled
    transposed bf16 weights, emitted per half (halves=2) so they overlap
    the second half of each layer's groups.

HBM traffic per core: ~31.9 MB (vs 42 MB v2, 168 MB plain fp32).

Scale ledger (host <-> device):
  K8 = e4(16*K), V8 = e4(16*V), q0_8 = e4(16*q0), w8 = e4(exp(logit)) ~ 1
  (unnormalized; 1/Z folded into the extraction via scalar_tensor_tensor)
  attn_dev = 16*attn ; W0h = (100/16)*W0^T -> q1_dev = 100*q1 (same for
  q2); Wqh = Wq^T -> qf_dev = 100*qf.
  exp scales: cs0 = 1/(16*16*sqrt(32)), cs1 = 1/(100*16*sqrt(32));
  tanh scale: ct = 1/(100*16*sqrt(512)).
"""

import sys

if "/opt/trn_rl_repo" not in sys.path:
    sys.path.insert(0, "/opt/trn_rl_repo")

import numpy as np

import concourse.bass as bass
import concourse.tile as tile
import concourse.mybir as mybir

F32 = mybir.dt.float32
BF16 = mybir.dt.bfloat16
FP8 = mybir.dt.float8e4

N_CORES = 8
B = 256
N = 512
D = 512
H = 16
DH = 32
DC = 4                # d chunks of 128
NPAD0 = 416           # group-0 capacity (13*32)
NPADR = 384           # groups 1+ capacity (3*128)
CLIP = 10.0

SK = 16.0             # fp8 scale for K, V, q0
SQ = 100.0            # device scale of q1/q2/qf
CS0 = 1.0 / (SK * SK * np.sqrt(DH))
CS1 = 1.0 / (SQ * SK * np.sqrt(DH))
CT = 1.0 / (SQ * SK * np.sqrt(D))


def _hoist_excess_matmul_waits(nc, keep=1):
    """walrus limits self-loading 4-byte matmuls (fp32/fp32r/transpose) to a
    single sync wait on the S3_LW struct.  Hoist excess waits onto a
    standalone PE EventSemaphore inserted right before the matmul — same
    engine, so per-engine program order makes it equivalent."""
    for fn in nc.m.functions:
        for blk in fn.blocks:
            il = blk.instructions
            i = 0
            while i < len(il):
                inst = il[i]
                si = inst.sync_info
                if (type(inst).__name__ != "InstEventSemaphore"
                        and si is not None
                        and si.on_wait and len(si.on_wait) > keep):
                    moved = list(si.on_wait[:-keep]) if keep else list(si.on_wait)
                    kept = list(si.on_wait[-keep:]) if keep else []
                    for j, w in enumerate(moved):
                        wi = mybir.InstEventSemaphore(
                            name=f"{inst.name}-hw{j}",
                            ins=[], outs=[],
                            sync_info=mybir.SyncInfo(on_wait=[w], on_update=[]),
                        )
                        wi.engine = inst.engine
                        nc.register_instruction(wi)
                        il.insert(i, wi)
                        i += 1
                    inst.sync_info = mybir.SyncInfo(
                        on_wait=kept, on_update=list(si.on_update)
                    )
                i += 1


def build_nc(b_core=32, npad0=NPAD0, npadr=NPADR, reps=1, dma_only=False,
             kv_bufs=7, k2_bufs=6, compute_only=False, staggered=False,
             unroll=False, dve_t=False, halves=2, ps_bufs=3, pz_bufs=1,
             cast_mode="dve", l2i=False):
    """Build the single-core Bass program for a [b_core]-batch shard with
    group-0 n-capacity npad0 (tail = npad0 % 128 via a resident V-tail
    tile) and npadr for the remaining groups (npadr % 128 must be 0,
    or npadr == npad0)."""
    groups = b_core // 4
    nfull0 = npad0 // 128
    ntail0 = npad0 - 128 * nfull0     # 0 or a multiple of 32
    nfullr = npadr // 128
    assert npadr == npad0 or npadr % 128 == 0
    assert nfull0 == nfullr or ntail0 == 0
    nc = bass.Bass()

    def np_g(g):
        return npad0 if g == 0 else npadr

    # K^T chunks (idx 4k+e: partition p = d within chunk e of batch k,
    # free = compacted n); group 0 vs rest have different n capacity
    kt0 = nc.declare_dram_parameter("kt0", [2, 128, 16, npad0], FP8,
                                    isOutput=False)
    ktr = nc.declare_dram_parameter("ktr", [2 * (groups - 1), 128, 16, npadr],
                                    FP8, isOutput=False)
    # V full chunks (idx k*nfull+c: partition p = n within chunk c of
    # batch k, free = d); same shape for both classes (nfull equal)
    v0 = nc.declare_dram_parameter("v0", [2, 128, 4 * nfull0, D], FP8,
                                   isOutput=False)
    vr = nc.declare_dram_parameter("vr", [2 * (groups - 1), 128, 4 * nfullr, D],
                                   FP8, isOutput=False)
    if ntail0:
        vt = nc.declare_dram_parameter("vt", [ntail0, 2, 4, D], FP8,
                                       isOutput=False)
    k20 = nc.declare_dram_parameter("k20", [128, 16, npad0], FP8,
                                    isOutput=False)
    k2r = nc.declare_dram_parameter("k2r", [groups - 1, 128, 16, npadr], FP8,
                                    isOutput=False)
    qbd0 = nc.declare_dram_parameter("qbd0", [128, DC, b_core * 32], FP8,
                                     isOutput=False)
    w0t = nc.declare_dram_parameter("w0t", [128, DC, D], BF16, isOutput=False)
    wqt = nc.declare_dram_parameter("wqt", [128, DC, D], BF16, isOutput=False)
    b0t = nc.declare_dram_parameter("b0t", [128, DC], F32, isOutput=False)
    bqt = nc.declare_dram_parameter("bqt", [128, DC], F32, isOutput=False)
    nzc = nc.declare_dram_parameter("nzc", [128, groups], F32, isOutput=False)
    nzc2 = nc.declare_dram_parameter("nzc2", [128, groups], F32, isOutput=False)
    slotsel = nc.declare_dram_parameter("slotsel", [128, 4], BF16, isOutput=False)
    sely = nc.declare_dram_parameter("sely", [128, D], F32, isOutput=False)
    ident = nc.declare_dram_parameter("ident", [128, 128], F32, isOutput=False)
    out = nc.declare_dram_parameter("out", [b_core, npad0], F32, isOutput=True)

    with tile.TileContext(nc) as tc:
        with (
            tc.tile_pool(name="singles", bufs=1) as singles,
            tc.tile_pool(name="kvpool", bufs=kv_bufs) as kvpool,
            tc.tile_pool(name="k2pool", bufs=k2_bufs) as k2pool,
            tc.tile_pool(name="work", bufs=3) as work,
            tc.tile_pool(name="small", bufs=8) as small,
            tc.tile_pool(name="p_s", bufs=ps_bufs, space="PSUM") as p_s,
            tc.tile_pool(name="p_y", bufs=2, space="PSUM") as p_y,
            tc.tile_pool(name="p_wt", bufs=1, space="PSUM") as p_wt,
            tc.tile_pool(name="p_zt", bufs=pz_bufs, space="PSUM") as p_zt,
            tc.tile_pool(name="p_q", bufs=1, space="PSUM") as p_q,
        ):
            # ---- constants / weights ----
            sb_qbd = singles.tile([128, DC, b_core * 32], FP8)
            nc.sync.dma_start(sb_qbd[:], qbd0[:])
            if ntail0:
                sb_vt = singles.tile([ntail0, 2, 4, D], FP8)
                nc.sync.dma_start(sb_vt[:], vt[:])
            sb_w0t = singles.tile([128, DC, D], BF16)
            nc.sync.dma_start(sb_w0t[:], w0t[:])
            sb_wqt = singles.tile([128, DC, D], BF16)
            nc.sync.dma_start(sb_wqt[:], wqt[:])
            sb_b0t = singles.tile([128, DC], F32)
            nc.sync.dma_start(sb_b0t[:], b0t[:])
            sb_bqt = singles.tile([128, DC], F32)
            nc.sync.dma_start(sb_bqt[:], bqt[:])
            sb_nzc = singles.tile([128, groups], F32)
            nc.sync.dma_start(sb_nzc[:], nzc[:])
            sb_nzc2 = singles.tile([128, groups], F32)
            nc.sync.dma_start(sb_nzc2[:], nzc2[:])
            sb_slotsel = singles.tile([128, 4], BF16)
            nc.sync.dma_start(sb_slotsel[:], slotsel[:])
            sb_selyf = singles.tile([128, D], F32)
            nc.sync.dma_start(sb_selyf[:], sely[:])
            sb_ident = singles.tile([128, 128], F32)
            nc.sync.dma_start(sb_ident[:], ident[:])
            sb_nclip = singles.tile([128, 1], F32)
            nc.vector.memset(sb_nclip[:], -CLIP)
            sb_identb = singles.tile([128, 128], BF16)
            nc.vector.tensor_copy(sb_identb[:], sb_ident[:])

            # qf_pad zeroing is off the critical path: do it first (once —
            # unrolled reps rewrite the same lanes each trip)
            qf_pad = singles.tile([128, DC, b_core * 32], FP8)
            nc.vector.memset(qf_pad[:], 0.0)
            qf_v = qf_pad.rearrange("p e (b j) -> p e b j", j=32)

            def softmax_e(ps_s, cs, g, npg):
                """psum scores [128, npg] -> unnormalized e [128, npg] SBUF
                + rz = 1/sum(e) over legal slots; pad slots contribute
                exp(0)=1 each, subtracted exactly via nzc."""
                zsum = small.tile([128, 1], F32, tag="zsum")
                e_t = work.tile([128, npad0], BF16, tag="e_t")
                nc.scalar.activation(
                    e_t[:, :npg], ps_s[:, :npg],
                    mybir.ActivationFunctionType.Exp,
                    scale=cs, accum_out=zsum[:],
                )
                zc = small.tile([128, 1], F32, tag="zc")
                nc.vector.tensor_scalar_add(zc[:], zsum[:], sb_nzc[:, g:g + 1])
                rz = small.tile([128, 1], F32, tag="rz")
                nc.vector.reciprocal(rz[:], zc[:])
                return e_t, rz

            def projection(attn_h, wt, bt, tag, hb, out_dt=F32):
                """q_nextT [128, DC(j), hb] = W^T @ attn^T + bias, for a
                half-layer batch slice (hb batches)."""
                ps_q = p_q.tile([128, DC, hb], F32, tag="ps_q")
                for jc in range(DC):
                    for ic in range(DC):
                        nc.tensor.matmul(
                            ps_q[:, jc, :],
                            wt[:, ic, 128 * jc:128 * jc + 128],
                            attn_h[:, ic, :],
                            start=(ic == 0), stop=(ic == DC - 1),
                        )
                qt = work.tile([128, DC, hb], out_dt, tag=tag)
                for jc in range(DC):
                    nc.vector.tensor_scalar_add(
                        qt[:, jc, :], ps_q[:, jc, :], bt[:, jc:jc + 1]
                    )
                return qt

            def fill_qbd_half(qt, h, hb):
                """Overwrite the block-diagonal of sb_qbd for batches
                [h*hb, (h+1)*hb) from qt [128, DC, hb] (f32 -> fp8 cast;
                split across DVE and ACT)."""
                qbd_v = sb_qbd.rearrange("p e (b j) -> p e b j", j=32)
                for e in range(DC):
                    for g in range(4):
                        dst = qbd_v[32 * g:32 * g + 32, e,
                                    h * hb:(h + 1) * hb, 4 * e + g]
                        src = qt[32 * g:32 * g + 32, e, :]
                        if (e + g) % 2 == 0:
                            nc.vector.tensor_copy(dst, src)
                        else:
                            nc.scalar.copy(dst, src)

            def _emit_body():
                gph = groups // halves          # groups per half
                hb = b_core // halves           # batches per half

                def boundary(l, attn_h, h):
                    """Projection + query-constant fill for half h of layer l;
                    emitted as soon as that half's groups are done."""
                    if l == 0:
                        qt1 = projection(attn_h, sb_w0t, sb_b0t, "qt1", hb)
                        fill_qbd_half(qt1, h, hb)
                    else:
                        q2t = projection(attn_h, sb_w0t, sb_b0t, "qt2", hb,
                                         out_dt=BF16)
                        qft = projection(q2t, sb_wqt, sb_bqt, "qft", hb)
                        for e in range(DC):
                            nc.vector.tensor_copy(
                                qf_v[:, e, h * hb:(h + 1) * hb, 0],
                                qft[:, e, :],
                            )

                # ================= layers 0, 1 =================
                for l in range(2):
                    cs = CS0 if l == 0 else CS1
                    attn_hs = [work.tile([128, DC, hb], BF16, tag=f"attn{h}",
                                         name=f"attn{h}")
                               for h in range(halves)]
                    psat_hs = [p_zt.tile([128, DC, hb], F32, tag="ps_at",
                                         name=f"psat{h}")
                               for h in range(halves)]
                    for g in range(groups):
                        npg = np_g(g)
                        nfg = nfull0 if g == 0 else nfullr
                        ntg = ntail0 if g == 0 else 0
                        attn_sb = attn_hs[g // gph]
                        gh = g % gph            # group index within the half
                        if compute_only:
                            if g == 0 and l == 0:
                                kt_c = singles.tile([128, 16, npad0], FP8)
                                nc.sync.dma_start(kt_c[:], kt0[0])
                                v_c = singles.tile([128, 4 * nfull0, D], FP8)
                                nc.sync.dma_start(v_c[:], v0[0])
                            kt, v = kt_c, v_c
                        elif g == 0:
                            kt = kvpool.tile([128, 16, npad0], FP8, tag="kt0",
                                             bufs=2)
                            nc.sync.dma_start(kt[:], kt0[l])
                            v = kvpool.tile([128, 4 * nfull0, D], FP8,
                                            tag="v0", bufs=2)
                            nc.sync.dma_start(v[:], v0[l])
                        else:
                            kt = kvpool.tile([128, 16, npadr], FP8, tag="ktr")
                            nc.sync.dma_start(kt[:],
                                              ktr[l * (groups - 1) + g - 1])
                            v = kvpool.tile([128, 4 * nfullr, D], FP8,
                                            tag="vr")
                            nc.sync.dma_start(v[:],
                                              vr[l * (groups - 1) + g - 1])
                        # scores: 4 batches col-tiled, wave-major over e
                        ps_s = p_s.tile([128, npad0], F32, tag="ps_s")
                        for e in range(DC):
                            for k in range(4):
                                b = 4 * g + k
                                nc.tensor.matmul(
                                    ps_s[32 * k:32 * k + 32, :npg],
                                    sb_qbd[:, e, 32 * b:32 * b + 32],
                                    kt[:, 4 * k + e, :],
                                    start=(e == 0),
                                    stop=(e == DC - 1),
                                    tile_position=(0, 32 * k),
                                )
                        e_t, rz = softmax_e(ps_s, cs, g, npg)
                        # w'^T: PE transposes + ACT fp8 cast, or DVE
                        # square-block transposes (bf16)
                        if dve_t:
                            wt8 = work.tile([128, 4, 128], BF16, tag="wt8")
                            for c in range(nfg):
                                nc.vector.transpose(
                                    wt8[:, c, :], e_t[:, 128 * c:128 * c + 128]
                                )
                            for t in range(ntg // 32):
                                for k in range(4):
                                    nc.vector.transpose(
                                        wt8[32 * t:32 * t + 32, nfg,
                                            32 * k:32 * k + 32],
                                        e_t[32 * k:32 * k + 32,
                                            128 * nfg + 32 * t:
                                            128 * nfg + 32 * t + 32],
                                    )
                        else:
                            pwt = p_wt.tile([128, 4, 128], BF16, tag="pwt")
                            for c in range(nfg):
                                nc.tensor.transpose(
                                    pwt[:, c, :], e_t[:, 128 * c:128 * c + 128],
                                    sb_identb[:]
                                )
                            if ntg:
                                nc.tensor.transpose(
                                    pwt[:ntg, nfg, :],
                                    e_t[:, 128 * nfg:npg],
                                    sb_identb[:]
                                )
                            wt8 = work.tile([128, 4, 128], FP8, tag="wt8")
                            use_dve = (cast_mode == "dve"
                                       or (cast_mode == "alt" and g % 2))
                            cp = (nc.vector.tensor_copy if use_dve
                                  else nc.scalar.copy)
                            cp(wt8[:, :nfg, :], pwt[:, :nfg, :])
                            if ntg:
                                cp(wt8[:ntg, nfg, :], pwt[:ntg, nfg, :])
                        # AV y-form: 4 batches col-tiled, wave-major over c
                        ps_y = p_y.tile([128, D], F32, tag="ps_y")
                        for c in range(nfg):
                            for k in range(4):
                                nc.tensor.matmul(
                                    ps_y[32 * k:32 * k + 32, :],
                                    wt8[:, c, 32 * k:32 * k + 32],
                                    v[:, k * nfg + c, :],
                                    start=(c == 0),
                                    stop=(ntg == 0 and c == nfg - 1),
                                    tile_position=(0, 32 * k),
                                )
                        if ntg:
                            for k in range(4):
                                nc.tensor.matmul(
                                    ps_y[32 * k:32 * k + 32, :],
                                    wt8[:ntg, nfg, 32 * k:32 * k + 32],
                                    sb_vt[:, l, k, :],
                                    start=False, stop=True,
                                    tile_position=(0, 32 * k),
                                )
                        # head-diag extraction with fused 1/Z normalization:
                        # zt = (Y * rz) * selY  -> attn^T[:, :, b]
                        zt = work.tile([128, D], BF16, tag="zt")
                        nc.vector.scalar_tensor_tensor(
                            zt[:], ps_y[:], rz[:], sb_selyf[:],
                            op0=mybir.AluOpType.mult, op1=mybir.AluOpType.mult,
                        )
                        # slot-row sums as 4 tiny PE matmuls: zt chunk as
                        # stationary, 0/1 slot selector streaming ->
                        # attn^T[:, c, 4 batches] straight into PSUM
                        ps_at = psat_hs[g // gph]
                        for c in range(DC):
                            nc.tensor.matmul(
                                ps_at[:, c, 4 * gh:4 * gh + 4],
                                zt[:, 128 * c:128 * c + 128],
                                sb_slotsel[:],
                                start=True, stop=True,
                            )
                        if gh == gph - 1:
                            nc.scalar.copy(attn_sb[:], psat_hs[g // gph][:])
                            boundary(l, attn_sb, g // gph)
                            if l == 1 and l2i:
                                # layer-2 interleave: this half's qf is ready;
                                # emit its layer-2 groups now so they overlap
                                # the remaining layer-1 half / next-rep fill
                                h = g // gph
                                for g2 in range(h * gph, (h + 1) * gph):
                                    l2_group(g2)

                # ================= layer 2 =================
                if not l2i:
                    for g in range(groups):
                        l2_group(g)

            kt2_c = [None]

            def l2_group(g):
                    npg = np_g(g)
                    if compute_only:
                        if g == 0:
                            kt2_c[0] = singles.tile([128, 16, npad0], FP8)
                            nc.sync.dma_start(kt2_c[0][:], k20[:])
                        kt2 = kt2_c[0]
                    elif g == 0:
                        kt2 = k2pool.tile([128, 16, npad0], FP8, tag="kt20",
                                          bufs=1)
                        nc.sync.dma_start(kt2[:], k20[:])
                    else:
                        kt2 = k2pool.tile([128, 16, npadr], FP8, tag="kt2r")
                        nc.sync.dma_start(kt2[:], k2r[g - 1])
                    ps_s2 = p_s.tile([128, npad0], F32, tag="ps_s")
                    for e in range(DC):
                        for k in range(4):
                            b = 4 * g + k
                            nc.tensor.matmul(
                                ps_s2[32 * k:32 * k + 32, :npg],
                                qf_pad[:, e, 32 * b:32 * b + 32],
                                kt2[:, 4 * k + e, :],
                                start=(e == 0),
                                stop=(e == DC - 1),
                                tile_position=(0, 32 * k),
                            )
                    u_t = work.tile([128, npad0], F32, tag="u_t")
                    nc.scalar.activation(
                        u_t[:, :npg], ps_s2[:, :npg],
                        mybir.ActivationFunctionType.Tanh,
                        scale=CT,
                    )
                    zsum2 = small.tile([128, 1], F32, tag="zsum2")
                    e2_t = work.tile([128, npad0], F32, tag="e2_t")
                    nc.scalar.activation(
                        e2_t[:, :npg], u_t[:, :npg],
                        mybir.ActivationFunctionType.Exp,
                        bias=sb_nclip[:], scale=CLIP, accum_out=zsum2[:],
                    )
                    zc2 = small.tile([128, 1], F32, tag="zc2")
                    nc.vector.tensor_scalar_add(zc2[:], zsum2[:],
                                                sb_nzc2[:, g:g + 1])
                    rz2 = small.tile([128, 1], F32, tag="rz2")
                    nc.vector.reciprocal(rz2[:], zc2[:])
                    w2_t = work.tile([128, npad0], F32, tag="w2_t")
                    nc.vector.tensor_scalar_mul(w2_t[:, :npg], e2_t[:, :npg],
                                                rz2[:])
                    nc.sync.dma_start(
                        out[4 * g:4 * g + 4, :npg],
                        w2_t.rearrange("(k r) n -> k r n", r=32)[:, 0, :npg],
                    )

            def _emit_dma_only():
                """Same DMA stream as the real kernel; tiny DVE consumers keep
                each tile live. Measures the DMA floor."""
                sink = small.tile([128, 1], F32, tag="sink")
                for l in range(2):
                    for g in range(groups):
                        if g == 0:
                            kt = kvpool.tile([128, 16, npad0], FP8, tag="kt0",
                                             bufs=2)
                            nc.sync.dma_start(kt[:], kt0[l])
                            v = kvpool.tile([128, 4 * nfull0, D], FP8,
                                            tag="v0", bufs=2)
                            nc.sync.dma_start(v[:], v0[l])
                        else:
                            kt = kvpool.tile([128, 16, npadr], FP8, tag="ktr")
                            nc.sync.dma_start(kt[:],
                                              ktr[l * (groups - 1) + g - 1])
                            v = kvpool.tile([128, 4 * nfullr, D], FP8,
                                            tag="vr")
                            nc.sync.dma_start(v[:],
                                              vr[l * (groups - 1) + g - 1])
                        nc.vector.tensor_reduce(
                            sink[:], kt[:, 0, :4], axis=mybir.AxisListType.X,
                            op=mybir.AluOpType.add,
                        )
                        nc.vector.tensor_reduce(
                            sink[:], v[:, 0, :4], axis=mybir.AxisListType.X,
                            op=mybir.AluOpType.add,
                        )
                w2_t = work.tile([128, npad0], F32, tag="w2_t")
                nc.vector.memset(w2_t[:], 0.5)
                for g in range(groups):
                    if g == 0:
                        kt2 = k2pool.tile([128, 16, npad0], FP8, tag="kt20",
                                          bufs=1)
                        nc.sync.dma_start(kt2[:], k20[:])
                    else:
                        kt2 = k2pool.tile([128, 16, npadr], FP8, tag="kt2r")
                        nc.sync.dma_start(kt2[:], k2r[g - 1])
                    nc.vector.tensor_reduce(
                        sink[:], kt2[:, 0, :4], axis=mybir.AxisListType.X,
                        op=mybir.AluOpType.add,
                    )
                    nc.sync.dma_start(
                        out[4 * g:4 * g + 4, :],
                        w2_t.rearrange("(k r) n -> k r n", r=32)[:, 0, :],
                    )

            import contextlib
            if unroll and reps > 1:
                # Python-unrolled reps: no HW-loop barrier, so the Tile
                # scheduler pipelines across reps — steady-state throughput
                for r in range(reps):
                    if r:
                        nc.sync.dma_start(sb_qbd[:], qbd0[:])
                    if dma_only:
                        _emit_dma_only()
                    else:
                        _emit_body()
            else:
                loop_cm = (tc.For_i(0, reps, 1, staggered_reset=staggered)
                           if reps > 1 else contextlib.nullcontext())
                with loop_cm:
                    if reps > 1:
                        # re-load the block-diag query so each rep is identical
                        nc.sync.dma_start(sb_qbd[:], qbd0[:])
                    if dma_only:
                        _emit_dma_only()
                    else:
                        _emit_body()

    _hoist_excess_matmul_waits(nc)
    return nc


# ---------------- host-side preparation ----------------

def _host_constants():
    import ml_dtypes
    p = np.arange(128)
    # selY[p, d] = 1 iff (p % 32) == d // 32   (slot row p holds head p%32;
    # head h owns d in [32h, 32h+32); pad rows 16..31 never match)
    selY = ((p[:, None] % 32) == (np.arange(D)[None, :] // 32)).astype(np.float32)
    r = np.arange(4)
    slotsel = ((p[:, None] // 32) == r[None, :]).astype(ml_dtypes.bfloat16)
    ident = np.eye(128, dtype=np.float32)
    return selY, slotsel, ident


def _prep_weights(W0_w, W0_b, Wq_w, Wq_b):
    # attn_dev = SV*attn (w = e/Z exact f32, V at SV scale)
    s0 = SQ / SK
    import ml_dtypes
    w0t = (np.asarray(W0_w, np.float32).T * s0).reshape(DC, 128, D)
    w0t = np.ascontiguousarray(w0t.transpose(1, 0, 2)).astype(ml_dtypes.bfloat16)
    wqt = np.asarray(Wq_w, np.float32).T.reshape(DC, 128, D)
    wqt = np.ascontiguousarray(wqt.transpose(1, 0, 2)).astype(ml_dtypes.bfloat16)
    b0t = np.ascontiguousarray(
        (np.asarray(W0_b, np.float32) * SQ).reshape(DC, 128).T)
    bqt = np.ascontiguousarray(
        (np.asarray(Wq_b, np.float32) * SQ).reshape(DC, 128).T)
    return w0t, wqt, b0t, bqt


def _quant8(x):
    import ml_dtypes
    return np.asarray(x, np.float32).astype(ml_dtypes.float8_e4m3)


def global_perm(mask, npad0, npadr):
    """Deal batches so each core's group 0 holds its 4 largest legal-count
    batches.  Returns perm (kernel batch order: perm[i] = original batch
    index of kernel slot i) or None if the input does not fit."""
    cnt = (~mask).sum(axis=1)
    if cnt.max() > npad0:
        return None
    b_core = mask.shape[0] // N_CORES
    order = np.argsort(-cnt, kind="stable")
    # snake-deal the 8*4 largest into the cores' group-0 slots
    g0 = order[:4 * N_CORES].reshape(4, N_CORES)
    g0[1::2] = g0[1::2, ::-1]
    rest = np.sort(order[4 * N_CORES:])
    perm = np.empty(mask.shape[0], np.int64)
    for c in range(N_CORES):
        perm[c * b_core:c * b_core + 4] = g0[:, c]
        perm[c * b_core + 4:(c + 1) * b_core] = rest[c * (b_core - 4):
                                                     (c + 1) * (b_core - 4)]
    # every non-group-0 slot must fit in npadr
    if cnt[perm[np.arange(mask.shape[0]) % b_core >= 4]].max() > npadr:
        return None
    return perm


def _compact_idx(mask, caps):
    """Per-batch legal-rows-first gather index [b, max(caps)] + counts.
    caps[i] = that slot's capacity; pads repeat index 0 (K=V zeroed)."""
    b = mask.shape[0]
    mcap = int(max(caps))
    idx = np.zeros((b, mcap), np.int64)
    cnt = np.zeros((b,), np.int64)
    for i in range(b):
        legal = np.flatnonzero(~mask[i])
        assert len(legal) <= caps[i]
        cnt[i] = len(legal)
        idx[i, :len(legal)] = legal
    return idx, cnt


def _prep_kv_core(K_c, V_c, idx, cnt, b_core, npad0, npadr):
    """Build kt0/ktr, v0/vr, vt, k20/k2r (all fp8e4m3 at 16x scale) for one
    core's batch shard, rows compacted via idx; pad rows zeroed."""
    groups = b_core // 4
    nfull0 = npad0 // 128
    ntail0 = npad0 - 128 * nfull0
    nfullr = npadr // 128
    ar = np.arange(b_core)[:, None]
    mcap = idx.shape[1]
    pad = np.arange(mcap)[None, :] >= cnt[:, None]             # [b, mcap]
    Kg = np.asarray(K_c, np.float32)[ar, idx]                  # [b, mcap, 3D]
    Vg = np.asarray(V_c, np.float32)[ar, idx]
    Kg[pad] = 0.0
    Vg[pad] = 0.0
    K8 = _quant8(SK * Kg)
    V8 = _quant8(SK * Vg)

    def kt_of(K8s, npad):
        """[nb, npad, 3D] -> [3, nb//4, 128, 16, npad]"""
        nb = K8s.shape[0]
        Kv = K8s[:, :npad].reshape(nb // 4, 4, npad, 3, DC, 128)
        return np.ascontiguousarray(Kv.transpose(3, 0, 5, 1, 4, 2)).reshape(
            3, nb // 4, 128, 16, npad)

    def v_of(V8s, nfull):
        nb = V8s.shape[0]
        Vv = V8s[:, :128 * nfull].reshape(nb // 4, 4, nfull, 128, 3, D)
        return np.ascontiguousarray(Vv.transpose(4, 0, 3, 1, 2, 5)).reshape(
            3, nb // 4, 128, 4 * nfull, D)

    kt0_3 = kt_of(K8[:4], npad0)           # [3, 1, 128, 16, npad0]
    ktr_3 = kt_of(K8[4:], npadr)           # [3, G-1, 128, 16, npadr]
    v0_3 = v_of(V8[:4], nfull0)
    vr_3 = v_of(V8[4:], nfullr)
    kt0 = np.ascontiguousarray(kt0_3[:2, 0])                   # [2, 128, 16, npad0]
    ktr = np.ascontiguousarray(ktr_3[:2]).reshape(2 * (groups - 1), 128, 16,
                                                  npadr)
    v0 = np.ascontiguousarray(v0_3[:2, 0])
    vr = np.ascontiguousarray(vr_3[:2]).reshape(2 * (groups - 1), 128,
                                                4 * nfullr, D)
    k20 = np.ascontiguousarray(kt0_3[2, 0])
    k2r = np.ascontiguousarray(ktr_3[2])
    vt = None
    if ntail0:
        # vt[r, l, k, d] = V8[k, 128*nfull0 + r, l*D + d]
        Vt = V8[:4, 128 * nfull0:npad0].reshape(4, ntail0, 3, D)
        vt = np.ascontiguousarray(Vt.transpose(1, 2, 0, 3)[:, :2])
    return kt0, ktr, v0, vr, vt, k20, k2r


def _prep_core(query_c, cnt, b_core, npad0, npadr):
    """Per-core fp8 block-diag query + f32 Z-correction tensors."""
    groups = b_core // 4
    qs = SK * np.asarray(query_c[:, 0, :], np.float32)          # [b, D]
    qbd = np.zeros((128, DC, b_core, 32), np.float32)
    for e in range(DC):
        for g in range(4):
            # rows 32g..32g+32 of chunk e hold d = 128e + 32g .., head 4e+g
            qbd[32 * g:32 * g + 32, e, :, 4 * e + g] = qs[:, 128 * e + 32 * g:
                                                          128 * e + 32 * g + 32].T
    qbd8 = _quant8(qbd.reshape(128, DC, b_core * 32))
    # nzc[p, g] = -(npad_g - cnt[4g + p//32]); nzc2 scales by exp(-CLIP)
    caps = np.where(np.arange(b_core) < 4, npad0, npadr)
    padn = (caps - cnt).astype(np.float32).reshape(groups, 4)  # [g, k]
    nzc = -np.ascontiguousarray(
        np.repeat(padn, 32, axis=1).T).astype(np.float32)      # [128, g]
    nzc2 = np.float32(np.exp(-CLIP)) * nzc
    return qbd8, nzc, nzc2


def prep_in_maps(query, K_att, V_att, mask, W0_w, W0_b, Wq_w, Wq_b,
                 npad0, npadr, perm):
    """Build the 8 per-core input maps + the output scatter indices.
    perm is the kernel-order -> original-batch mapping (identity allowed)."""
    query = np.asarray(query, np.float32)[perm]
    K_att = np.asarray(K_att, np.float32)[perm]
    V_att = np.asarray(V_att, np.float32)[perm]
    mask = np.asarray(mask).astype(bool)[perm]
    b_core = B // N_CORES
    selY, slotsel, ident = _host_constants()
    w0t, wqt, b0t, bqt = _prep_weights(W0_w, W0_b, Wq_w, Wq_b)
    caps = np.where(np.arange(B) % b_core < 4, npad0, npadr)
    idx_all, cnt_all = _compact_idx(mask, caps)
    in_maps = []
    for i in range(N_CORES):
        sl = slice(i * b_core, (i + 1) * b_core)
        idx, cnt = idx_all[sl], cnt_all[sl]
        qbd8, nzc, nzc2 = _prep_core(query[sl], cnt, b_core, npad0, npadr)
        kt0, ktr, v0, vr, vt, k20, k2r = _prep_kv_core(
            K_att[sl], V_att[sl], idx, cnt, b_core, npad0, npadr)
        m = {
            "kt0": kt0, "ktr": ktr, "v0": v0, "vr": vr,
            "k20": k20, "k2r": k2r, "qbd0": qbd8,
            "w0t": w0t, "wqt": wqt, "b0t": b0t, "bqt": bqt,
            "nzc": nzc, "nzc2": nzc2,
            "slotsel": slotsel, "sely": selY, "ident": ident,
        }
        if vt is not None:
            m["vt"] = vt
        in_maps.append(m)
    return in_maps, idx_all, cnt_all


def scatter_out(res_out, idx_all, cnt_all, perm):
    """[B, npad0] compacted (permuted) weights -> [B, N] full output."""
    npad = res_out.shape[1]
    full = np.zeros((B, N + 1), np.float32)
    j = np.arange(npad)[None, :]
    tgt = np.where(j < cnt_all[:, None], idx_all[:, :npad], N)
    np.put_along_axis(full, tgt, np.asarray(res_out, np.float32), axis=1)
    unperm = np.empty(B, np.int64)
    unperm[perm] = np.arange(B)
    return full[unperm, :N]


_NC_CACHE = {}
TRACE = False          # test-harness hook: profile the run, fill LAST
LAST = {}


def plan_shapes(mask):
    """Choose capacities + batch permutation for this input."""
    mask = np.asarray(mask).astype(bool)
    perm = global_perm(mask, NPAD0, NPADR)
    if perm is not None:
        return NPAD0, NPADR, perm
    return N, N, np.arange(B)      # uncompacted fallback, always correct


def kernel(query, K_att, V_att, mask, W0_w, W0_b, Wq_w, Wq_b):
    from concourse.bass_utils import run_bass_kernel_spmd

    b_core = B // N_CORES
    npad0, npadr, perm = plan_shapes(mask)

    if (b_core, npad0, npadr) not in _NC_CACHE:
        _NC_CACHE[(b_core, npad0, npadr)] = build_nc(b_core, npad0=npad0,
                                                     npadr=npadr)
    nc = _NC_CACHE[(b_core, npad0, npadr)]

    in_maps, idx_all, cnt_all = prep_in_maps(
        query, K_att, V_att, mask, W0_w, W0_b, Wq_w, Wq_b,
        npad0, npadr, perm)

    rr = run_bass_kernel_spmd(nc, in_maps, list(range(N_CORES)), trace=TRACE)
    LAST["exec_time_ns"] = rr.exec_time_ns
    res = rr.results
    out_c = np.concatenate([res[i]["out"] for i in range(N_CORES)], axis=0)
    return scatter_out(out_c, idx_all, cnt_all, perm)
